# revision 26
# baseline (speedup 1.0000x reference)
"""Trainium2 Bass kernel for 3-layer GNN message passing with per-edge
multi-head attention over node history, distributed over 8 NeuronCores.

Sharding: nodes are relabeled by descending degree and dealt into
(superblock, core, slot) so that each 128-edge tile maps partition p <->
target slot p ("identity segment" scheme). Per-edge k/v history rows are
assembled on the host between launches (pure indexing) and streamed as
dense bf16.

Key structure (all FLOPs on device):
- segment-sum accumulates PSUM[slot, feat] with identity-stationary
  matmuls; per-edge messages are pre-combined on the DVE so each 128-edge
  tile costs one matmul (layer 1 tree-reduces whole chunks to one matmul).
- gcn_norm is separable (nrm_e = dinv_src * dinv_tgt): dinv_src rides in
  the dj8 stream / prescaled v-tables, dinv_tgt in the output activation
  scale, and the v-projection bias is restored after aggregation with a
  rank-1 (s/dinv x bv) matmul, using that attention weights sum to one.
- attention is in difference form: layer 2 needs only
  a0 = sigmoid(q . (k0-k1)); layer 3 uses e_t = exp(q . (k_t - k2)),
  1/Z = sigmoid(-ln(e0+e1)). k-projection biases cancel, streams are
  bias-free.
- next-layer tables are produced per-superblock inside the aggregation
  callback, so table building overlaps the edge stream instead of
  trailing it.
- everything is conjugated by the d-major head permutation so DVE ops are
  unit-stride bf16 (2x mode); each launch keeps <= 4 distinct ScalarE
  activation functions to avoid ACT_TABLE_LOAD thrash.
4 launches: proj, layer1, layer2, layer3+head.
"""

import sys
import types

import numpy as np
import ml_dtypes

sys.path.insert(0, "/opt/trn_rl_repo")

BF16 = ml_dtypes.bfloat16

# ---------------------------------------------------------------- fixups
_HOOK = [None]


def _install_fixups():
    if "antenv.axon_hooks" not in sys.modules:
        mod = types.ModuleType("antenv.axon_hooks")
        mod.set_axon_ntff_profile_hook = lambda h: _HOOK.__setitem__(0, h)
        mod.get_axon_ntff_profile_hook = lambda: _HOOK[0]
        sys.modules["antenv.axon_hooks"] = mod
        try:
            from trn_agent_boot.trn_boot import _ntff_profile_via_ctypes

            _HOOK[0] = _ntff_profile_via_ctypes("/opt/axon/libaxon_pjrt.so")
        except Exception:
            pass

    import concourse.tile as tile
    from concourse.vector_clock import ScopedClock
    import bass_rust

    if getattr(tile.TileContext, "_drain_split_installed", False):
        return

    def _drain_and_barrier(self, tick_clock, wait_clock):
        nc = self.nc
        drain_inst = nc.sync.drain()
        wait_clock.add_sem_waits(
            drain_inst.ins, ScopedClock({None: tick_clock.global_clock})
        )
        si = drain_inst.ins.sync_info
        waits = list(si.on_wait or []) if si is not None else []
        if len(waits) > 1:
            si.on_wait = waits[:1]
            for i in range(1, len(waits)):
                d2 = nc.sync.drain()
                d2.ins.sync_info = bass_rust.SyncInfo(
                    on_wait=waits[i : i + 1], on_update=[]
                )
        nc.all_engine_barrier()
        assert self.sems is not None
        popped = nc._tile_sem_poison_stack.pop()
        assert popped is self._sem_poison
        nc.clear_and_free_semaphores(list(self.sems.allocated().values()))
        nc.all_engine_barrier()

    tile.TileContext._drain_and_barrier = _drain_and_barrier
    tile.TileContext._drain_split_installed = True


# ---------------------------------------------------------------- constants
N = 20000
E = 320000
IN_C = 256
HID = 64
OUT_C = 64
HEADS = 8
DH = 8
NCORES = 8
NPC = N // NCORES  # 2500
SBT = 128  # target slots per superblock
NSB = (NPC + SBT - 1) // SBT  # 20 (last has 68 targets)
LASTW = NPC - (NSB - 1) * SBT  # 68
MAXG = 32  # max tiles per streamed chunk

# d-major permutation of the 64 features (8 heads x 8 dims), an involution
PRM = np.arange(HID).reshape(HEADS, DH).T.reshape(-1)

_CACHE = {}


# ---------------------------------------------------------------- host prep
def _preprocess(edge_index):
    row = np.asarray(edge_index[0], dtype=np.int64)
    col = np.asarray(edge_index[1], dtype=np.int64)
    loop = np.arange(N, dtype=np.int64)
    row_all = np.concatenate([row, loop])
    col_all = np.concatenate([col, loop])
    deg = np.bincount(col_all, minlength=N).astype(np.int64)
    dinv = (1.0 / np.sqrt(np.maximum(deg, 1))).astype(np.float32)
    nrm = (dinv[row_all] * dinv[col_all]).astype(np.float32)
    s_all = np.bincount(col_all, weights=nrm.astype(np.float64), minlength=N)
    s_all = s_all.astype(np.float32)

    # degree-sorted relabeling: rank r -> (superblock b, core c, slot p)
    order = np.argsort(-deg, kind="stable")  # global ids by desc degree
    b_of = np.empty(N, np.int64)
    c_of = np.empty(N, np.int64)
    p_of = np.empty(N, np.int64)
    ranks = np.arange(N)
    full = (NSB - 1) * 1024  # ranks dealt in blocks of 8*128
    b_of[ranks < full] = ranks[ranks < full] // 1024
    c_of[ranks < full] = (ranks[ranks < full] % 1024) // SBT
    p_of[ranks < full] = ranks[ranks < full] % SBT
    tail = ranks >= full
    b_of[tail] = NSB - 1
    c_of[tail] = (ranks[tail] - full) // LASTW
    p_of[tail] = (ranks[tail] - full) % LASTW
    # per-node placement (indexed by global id)
    nb = np.empty(N, np.int64); nb[order] = b_of
    ncr = np.empty(N, np.int64); ncr[order] = c_of
    npp = np.empty(N, np.int64); npp[order] = p_of
    # ids[c][b*128+p] = global id owned by core c at local index
    ids = np.empty((NCORES, NPC), np.int64)
    loc = nb * SBT + npp  # local index within core
    ids[ncr, loc] = np.arange(N)

    # tiles per superblock = max degree within the superblock (desc sorted)
    tps = np.zeros(NSB, np.int64)
    for b in range(NSB):
        r0 = b * 1024 if b < NSB - 1 else full
        tps[b] = max(1, int(deg[order[r0]]))
    sb_start = np.zeros(NSB + 1, np.int64)
    sb_start[1:] = np.cumsum(tps)
    tt = int(sb_start[-1])

    # scatter edges: edge i (sorted by target) lands at
    # core c(t), row p(t), column sb_start[b(t)] + within-target-rank
    es = np.argsort(col_all, kind="stable")
    tgt = col_all[es]
    src = row_all[es]
    start_of = np.zeros(N + 1, np.int64)
    start_of[1:] = np.cumsum(np.bincount(tgt, minlength=N))
    rank_in_tgt = np.arange(len(tgt)) - start_of[tgt]
    dcol = sb_start[nb[tgt]] + rank_in_tgt
    drow = npp[tgt]
    dcore = ncr[tgt]

    dinv_ext = np.concatenate([dinv, np.zeros(1, np.float32)])
    metas = []
    for c in range(NCORES):
        m = dcore == c
        eidx = np.full((128, tt), N, np.int64)  # sentinel -> zero row
        eidx[drow[m], dcol[m]] = src[m]
        dj = dinv_ext[eidx]  # [128, tt] f32, 0 at padding
        dj8 = np.ascontiguousarray(
            np.broadcast_to(dj[:, :, None], (128, tt, 8))
        ).astype(BF16)
        metas.append(dict(eidx=eidx, dj8=dj8))

    # chunk plan: per sb, tiles split into chunks of <= MAXG; sbs processed
    # smallest-first so the pipeline ramps quickly
    chunks = []  # (sb, t0, gw, first, last)
    for b in range(NSB - 1, -1, -1):
        t0 = int(sb_start[b])
        left = int(tps[b])
        while left > 0:
            gw = min(MAXG, left)
            chunks.append(
                (b, t0, gw, t0 == int(sb_start[b]), left == gw)
            )
            t0 += gw
            left -= gw
    return metas, tuple(int(x) for x in tps), tt, chunks, s_all, dinv, ids


_WS_CTR = [0]


def _split_multi_waits(nc, maxw=1):
    """This container's walrus rejects instructions with more than one sync
    wait; hoist excess waits onto NoOps inserted before the instruction."""
    from concourse import mybir

    for f in nc.m.functions:
        for bb in f.blocks:
            insts = list(bb.instructions)
            out = []
            changed = False
            for inst in insts:
                si = inst.sync_info
                waits = list(si.on_wait) if (si is not None and si.on_wait) else []
                if len(waits) > maxw:
                    excess = waits[: len(waits) - maxw]
                    for j in range(0, len(excess), maxw):
                        _WS_CTR[0] += 1
                        out.append(
                            mybir.InstNoOp(
                                name=f"waitsplit_{_WS_CTR[0]}",
                                engine=inst.engine,
                                sync_info=mybir.SyncInfo(
                                    on_wait=excess[j : j + maxw], on_update=[]
                                ),
                                bass_nofuse=True,
                            )
                        )
                    si.on_wait = waits[len(waits) - maxw :]
                    changed = True
                out.append(inst)
            if changed:
                bb.instructions = out


def _mk_nc():
    import concourse.bass as bass

    return bass.Bass(num_devices=NCORES, debug=False, target_bir_lowering=False)


def _load_w(nc, pool, dram_ap, p, f, tag, dtype=None):
    from concourse import mybir

    t = pool.tile([p, f], dtype or mybir.dt.float32, tag=tag)
    nc.sync.dma_start(t[:], dram_ap[:])
    return t


def _proj_sbuf(nc, w_t, src, dst, psum_pool):
    """dst[64, NPC] (SBUF) = w.T @ src, bias-free, bf16, 500-col chunks."""
    from concourse import mybir

    f32 = mybir.dt.float32
    Ident = mybir.ActivationFunctionType.Identity
    NCH = 500
    for j0 in range(0, NPC, NCH):
        w = min(NCH, NPC - j0)
        ps = psum_pool.tile([HID, 512], f32, tag="mt")
        nc.tensor.matmul(
            out=ps[:, :w], lhsT=w_t[:], rhs=src[:, j0 : j0 + w],
            start=True, stop=True,
        )
        nc.scalar.activation(dst[:, j0 : j0 + w], ps[:, :w], Ident)


def _consts(nc, tc, ctx):
    from concourse import mybir
    from concourse.masks import make_identity

    cpool = ctx.enter_context(tc.tile_pool(name="const", bufs=1))
    ident_f = cpool.tile([128, 128], mybir.dt.float32, tag="idf")
    make_identity(nc, ident_f[:])
    ident_b = cpool.tile([128, 128], mybir.dt.bfloat16, tag="idb")
    nc.vector.tensor_copy(ident_b[:], ident_f[:])
    return ident_f, ident_b


def _qrows_from_cols(nc, tc, ctx, qT_d, ident_f):
    """Load q column-table [64, NPC] f32, transpose per superblock into
    qrows [128, NSB, 64] bf16 (row p = q of slot p; pad slots zeroed)."""
    from concourse import mybir

    f32 = mybir.dt.float32
    bf = mybir.dt.bfloat16
    qpool = ctx.enter_context(tc.tile_pool(name="q", bufs=1))
    qT = qpool.tile([HID, NPC], f32, tag="qT")
    nc.sync.dma_start(qT[:], qT_d[:])
    qrows = qpool.tile([128, NSB, HID], bf, tag="qrows")
    nc.vector.memset(qrows[:], 0.0)
    with tc.tile_pool(name="pqt", bufs=2, space="PSUM") as pst:
        for b in range(NSB):
            j0 = b * SBT
            w = min(SBT, NPC - j0)
            ps = pst.tile([128, HID], f32, tag="qtp")
            nc.tensor.transpose(
                out=ps[:w], in_=qT[:, j0 : j0 + w], identity=ident_f[:HID, :HID]
            )
            nc.scalar.copy(qrows[:w, b], ps[:w])
    return qrows



def _plan_super(tps, ch, first_ch=8):
    """Fixed contiguous column windows walked high->low; each knows its
    per-superblock segments (descending b). The first window is small so
    the edge pipeline starts computing before the bulk stream lands.
    Returns list of (c0, gw, segs) with segs = (b, off, w, first, last)."""
    sb_start = [0]
    for t in tps:
        sb_start.append(sb_start[-1] + t)
    tt = sb_start[-1]
    out = []
    c1 = tt
    while c1 > 0:
        w = first_ch if not out else (min(32, ch) if len(out) == 1 else ch)
        c0 = max(0, c1 - w)
        segs = []
        for b in range(NSB - 1, -1, -1):
            lo = max(sb_start[b], c0)
            hi = min(sb_start[b + 1], c1)
            if hi <= lo:
                continue
            segs.append(
                (b, lo - c0, hi - lo, sb_start[b + 1] <= c1, sb_start[b] >= c0)
            )
        out.append((c0, c1 - c0, segs))
        c1 = c0
    return out


def _sb_w(b):
    j0 = b * SBT
    return j0, min(SBT, NPC - j0)


# ---------------------------------------------------------------- launch A
def _build_launch_A():
    import concourse.tile as tile
    from concourse import mybir
    from contextlib import ExitStack

    f32 = mybir.dt.float32
    bf = mybir.dt.bfloat16
    nc = _mk_nc()
    xT = nc.dram_tensor("xT", [IN_C, NPC], bf, kind="ExternalInput").ap()
    w1 = nc.dram_tensor("w1", [IN_C, HID], bf, kind="ExternalInput").ap()
    b1 = nc.dram_tensor("b1", [HID, 1], f32, kind="ExternalInput").ap()
    hT_out = nc.dram_tensor("hT_out", [HID, NPC], bf, kind="ExternalOutput").ap()

    with tile.TileContext(nc) as tc, ExitStack() as ctx:
        wpool = ctx.enter_context(tc.tile_pool(name="w", bufs=1))
        xpool = ctx.enter_context(tc.tile_pool(name="x", bufs=1))
        hpool = ctx.enter_context(tc.tile_pool(name="h", bufs=1))
        psum_pool = ctx.enter_context(tc.tile_pool(name="ps", bufs=2, space="PSUM"))

        w1a = _load_w(nc, wpool, w1[0:128, :], 128, HID, "w1a", bf)
        w1b = _load_w(nc, wpool, w1[128:256, :], 128, HID, "w1b", bf)
        b1t = _load_w(nc, wpool, b1, HID, 1, "b1t")
        xa = xpool.tile([128, NPC], bf, tag="xa")
        xb = xpool.tile([128, NPC], bf, tag="xb")
        nc.sync.dma_start(xa[:], xT[0:128, :])
        nc.sync.dma_start(xb[:], xT[128:256, :])

        hT = hpool.tile([HID, NPC], bf)
        NCH = 500
        Relu = mybir.ActivationFunctionType.Relu
        for j0 in range(0, NPC, NCH):
            w = min(NCH, NPC - j0)
            ps = psum_pool.tile([HID, NCH], f32, tag="p1")
            nc.tensor.matmul(out=ps[:, :w], lhsT=w1a[:], rhs=xa[:, j0 : j0 + w], start=True, stop=False)
            nc.tensor.matmul(out=ps[:, :w], lhsT=w1b[:], rhs=xb[:, j0 : j0 + w], start=False, stop=True)
            nc.scalar.activation(hT[:, j0 : j0 + w], ps[:, :w], Relu, bias=b1t[:])
            nc.gpsimd.dma_start(hT_out[:, j0 : j0 + w], hT[:, j0 : j0 + w])
    _split_multi_waits(nc)
    return nc


# ---------------------------------------------------------------- launch B (layer 1)
def _build_launch_B(tt, chunks):
    import concourse.tile as tile
    from concourse import mybir
    from contextlib import ExitStack

    f32 = mybir.dt.float32
    bf = mybir.dt.bfloat16
    AT = mybir.AluOpType
    Relu = mybir.ActivationFunctionType.Relu
    Ident = mybir.ActivationFunctionType.Identity
    nc = _mk_nc()

    ed_d = nc.dram_tensor("ed", [128, tt, HID], bf, kind="ExternalInput").ap()
    dj8_d = nc.dram_tensor("dj8", [128, tt, 8], bf, kind="ExternalInput").ap()
    hT_d = nc.dram_tensor("hT", [HID, NPC], bf, kind="ExternalInput").ap()
    dinv_nm_d = nc.dram_tensor("dinv_nm", [128, NSB], f32, kind="ExternalInput").ap()
    s_row_d = nc.dram_tensor("s_row", [1, NPC], bf, kind="ExternalInput").ap()
    bv0r_d = nc.dram_tensor("bv0r", [1, HID], bf, kind="ExternalInput").ap()
    wv0_d = nc.dram_tensor("wv0", [HID, HID], bf, kind="ExternalInput").ap()
    wk2_d = nc.dram_tensor("wk2", [HID, HID], bf, kind="ExternalInput").ap()
    wv2_d = nc.dram_tensor("wv2", [HID, HID], bf, kind="ExternalInput").ap()
    wq2_d = nc.dram_tensor("wq2", [HID, HID], bf, kind="ExternalInput").ap()
    bq2_d = nc.dram_tensor("bq2", [HID, 1], f32, kind="ExternalInput").ap()
    o1T_d = nc.dram_tensor("o1T", [HID, NPC], bf, kind="ExternalOutput").ap()
    kd_d = nc.dram_tensor("kd", [HID, NPC], bf, kind="ExternalOutput").ap()
    dv_d = nc.dram_tensor("dv", [HID, NPC], bf, kind="ExternalOutput").ap()
    v1s_d = nc.dram_tensor("v1s", [NPC, HID], bf, kind="ExternalOutput").ap()
    q2T_d = nc.dram_tensor("q2T", [HID, NPC], f32, kind="ExternalOutput").ap()

    with tile.TileContext(nc) as tc, ExitStack() as ctx:
        ident_f, ident_b = _consts(nc, tc, ctx)
        ed_pool = ctx.enter_context(tc.tile_pool(name="ed", bufs=3))
        pref = []
        for (b, t0, gw, first, last) in chunks[:1]:
            edp = ed_pool.tile([128, MAXG, HID], bf, tag="ed")
            nc.gpsimd.dma_start(edp[:, :gw], ed_d[:, t0 : t0 + gw, :])
            pref.append(edp)
        meta_pool = ctx.enter_context(tc.tile_pool(name="meta", bufs=1))
        dj8_t = meta_pool.tile([128, tt, 8], bf, tag="dj8")
        nc.sync.dma_start(dj8_t[:], dj8_d[:])
        wpool = ctx.enter_context(tc.tile_pool(name="w", bufs=1))
        hpool = ctx.enter_context(tc.tile_pool(name="h", bufs=1))
        tr_pool = ctx.enter_context(tc.tile_pool(name="tr", bufs=2))
        st_pool = ctx.enter_context(tc.tile_pool(name="st", bufs=2))
        tab_pool = ctx.enter_context(tc.tile_pool(name="tab", bufs=2))
        psum_seg = ctx.enter_context(tc.tile_pool(name="pseg", bufs=2, space="PSUM"))
        psum_t = ctx.enter_context(tc.tile_pool(name="pt", bufs=1, space="PSUM"))
        psum_m = ctx.enter_context(tc.tile_pool(name="pm", bufs=2, space="PSUM"))

        wv0t = _load_w(nc, wpool, wv0_d, HID, HID, "wv0t", bf)
        wk2t = _load_w(nc, wpool, wk2_d, HID, HID, "wk2t", bf)
        wv2t = _load_w(nc, wpool, wv2_d, HID, HID, "wv2t", bf)
        wq2t = _load_w(nc, wpool, wq2_d, HID, HID, "wq2t", bf)
        bq2t = _load_w(nc, wpool, bq2_d, HID, 1, "bq2t")
        dinv_nm = _load_w(nc, wpool, dinv_nm_d, 128, NSB, "dinv")
        s_row = _load_w(nc, wpool, s_row_d, 1, NPC, "srow", bf)
        bv0r = _load_w(nc, wpool, bv0r_d, 1, HID, "bv0r", bf)
        hT = hpool.tile([HID, NPC], bf, tag="hT")
        nc.sync.dma_start(hT[:], hT_d[:])
        o1T = hpool.tile([HID, NPC], bf, tag="o1T")

        # h-side layer-2 tables (independent of o1; overlaps edge ramp)
        khT = hpool.tile([HID, NPC], bf, tag="khT")
        _proj_sbuf(nc, wk2t, hT, khT, psum_m)
        vhT = hpool.tile([HID, NPC], bf, tag="vhT")
        _proj_sbuf(nc, wv2t, hT, vhT, psum_m)

        def group_tables(b):
            # tables for sbs [b, b+3] in one 512-col batch
            j0 = b * SBT
            wg = min(4 * SBT, NPC - j0)
            o1g = o1T[:, j0 : j0 + wg]
            nc.sync.dma_start(o1T_d[:, j0 : j0 + wg], o1g)
            ko1 = psum_m.tile([HID, 512], f32, tag="mt")
            nc.tensor.matmul(out=ko1[:, :wg], lhsT=wk2t[:], rhs=o1g, start=True, stop=True)
            kdc = tab_pool.tile([HID, 512], bf, tag="kdc")
            nc.vector.tensor_tensor(out=kdc[:, :wg], in0=khT[:, j0 : j0 + wg], in1=ko1[:, :wg], op=AT.subtract)
            nc.sync.dma_start(kd_d[:, j0 : j0 + wg], kdc[:, :wg])
            vo1 = psum_m.tile([HID, 512], f32, tag="mt")
            nc.tensor.matmul(out=vo1[:, :wg], lhsT=wv2t[:], rhs=o1g, start=True, stop=True)
            dvc = tab_pool.tile([HID, 512], bf, tag="dvc")
            nc.vector.tensor_tensor(out=dvc[:, :wg], in0=vhT[:, j0 : j0 + wg], in1=vo1[:, :wg], op=AT.subtract)
            nc.sync.dma_start(dv_d[:, j0 : j0 + wg], dvc[:, :wg])
            # v1s rows: dinv_j-scaled transposes of v1 = wv2.T o1
            v1g = tab_pool.tile([HID, 512], bf, tag="v1g")
            nc.scalar.copy(v1g[:, :wg], vo1[:, :wg])
            for k in range((wg + SBT - 1) // SBT):
                wk_ = min(SBT, wg - k * SBT)
                pv = psum_t.tile([128, HID], bf, tag="pv")
                nc.tensor.transpose(
                    out=pv[:wk_], in_=v1g[:, k * SBT : k * SBT + wk_],
                    identity=ident_b[:HID, :HID],
                )
                v1s = tab_pool.tile([128, HID], bf, tag="v1s")
                nc.scalar.activation(
                    v1s[:wk_], pv[:wk_], Ident, scale=dinv_nm[:wk_, b + k : b + k + 1]
                )
                nc.sync.dma_start(v1s_d[j0 + k * SBT : j0 + k * SBT + wk_, :], v1s[:wk_])
            # q2 columns
            q2 = psum_m.tile([HID, 512], f32, tag="mt")
            nc.tensor.matmul(out=q2[:, :wg], lhsT=wq2t[:], rhs=o1g, start=True, stop=True)
            q2s = tab_pool.tile([HID, 512], f32, tag="q2s")
            nc.scalar.activation(q2s[:, :wg], q2[:, :wg], Ident, bias=bq2t[:])
            nc.sync.dma_start(q2T_d[:, j0 : j0 + wg], q2s[:, :wg])

        def out_cb(b, psT):
            j0, w = _sb_w(b)
            st = st_pool.tile([128, HID], bf, tag="st")
            nc.scalar.activation(st[:w], psT[:w], Ident, scale=dinv_nm[:w, b : b + 1])
            pt = psum_t.tile([HID, 128], bf, tag="pt")
            nc.tensor.transpose(out=pt[:, :w], in_=st[:w], identity=ident_b[:w, :w])
            stT = st_pool.tile([HID, 128], bf, tag="stT")
            nc.scalar.copy(stT[:, :w], pt[:, :w])
            ps3 = psum_t.tile([HID, 128], f32, tag="ps3")
            nc.tensor.matmul(out=ps3[:, :w], lhsT=wv0t[:], rhs=stT[:, :w], start=True, stop=False)
            nc.tensor.matmul(
                out=ps3[:, :w], lhsT=bv0r[:], rhs=s_row[:, j0 : j0 + w],
                start=False, stop=True,
            )
            o1c = o1T[:, j0 : j0 + w]
            nc.scalar.activation(o1c, ps3[:, :w], Relu)
            if b % 4 == 0:
                group_tables(b)

        state = {"ps": None, "started": False, "pend_cb": None}

        def emit_mm(rhs, stop=False):
            nc.tensor.matmul(
                out=state["ps"][:], lhsT=ident_b[:], rhs=rhs,
                start=(not state["started"]), stop=stop,
            )
            state["started"] = True

        def process(b, t0, gw, first, last, ed_t, hs):
            if first:
                ps_new = psum_seg.tile([128, HID], f32, tag="ps")
                state["ps"] = ps_new
                state["started"] = False
            # tree-reduce the chunk's tiles; odd leftovers go straight to PE
            cur = hs
            width = gw
            level = 0
            mms = []
            while width > 1:
                if width % 2 == 1:
                    mms.append(cur[:, width - 1])
                    width -= 1
                half = width // 2
                nxt = tr_pool.tile([128, MAXG // 2, HID], bf, tag=f"tr{level}")
                nc.vector.tensor_tensor(
                    out=nxt[:, :half], in0=cur[:, 0:half], in1=cur[:, half:width],
                    op=AT.add,
                )
                cur = nxt
                width = half
                level += 1
            mms.append(cur[:, 0])
            for i, rhs in enumerate(mms):
                emit_mm(rhs, stop=(last and i == len(mms) - 1))
            if last:
                if state.get("pend_cb") is not None:
                    out_cb(*state["pend_cb"])
                state["pend_cb"] = (b, state["ps"])

        pend = None
        for ci, (b, t0, gw, first, last) in enumerate(chunks):
            if ci < 1:
                ed_t = pref[ci]
            else:
                ed_t = ed_pool.tile([128, MAXG, HID], bf, tag="ed")
                nc.gpsimd.dma_start(ed_t[:, :gw], ed_d[:, t0 : t0 + gw, :])
            hs = ed_pool.tile([128, MAXG, HID], bf, tag="hs")
            nc.vector.tensor_tensor(
                out=hs[:, :gw].rearrange("p c (d h) -> p c d h", d=8),
                in0=ed_t[:, :gw].rearrange("p c (d h) -> p c d h", d=8),
                in1=dj8_t[:, t0 : t0 + gw, None, :].to_broadcast([128, gw, 8, 8]),
                op=AT.mult,
            )
            if pend is not None:
                process(*pend)
            pend = (b, t0, gw, first, last, ed_t, hs)
        process(*pend)
        if state.get("pend_cb") is not None:
            out_cb(*state["pend_cb"])
    _split_multi_waits(nc)
    return nc


# ---------------------------------------------------------------- launch C (layer 2)
def _build_launch_C(tt, tps):
    import concourse.tile as tile
    from concourse import mybir
    from contextlib import ExitStack

    f32 = mybir.dt.float32
    bf = mybir.dt.bfloat16
    AT = mybir.AluOpType
    Relu = mybir.ActivationFunctionType.Relu
    Ident = mybir.ActivationFunctionType.Identity
    Sig = mybir.ActivationFunctionType.Sigmoid
    nc = _mk_nc()
    roww = 3 * HID  # 192: [kd | dv | v1s]
    CH = 64
    plan = _plan_super(tps, CH)

    ed_d = nc.dram_tensor("ed", [128, tt, roww], bf, kind="ExternalInput").ap()
    dj8_d = nc.dram_tensor("dj8", [128, tt, 8], bf, kind="ExternalInput").ap()
    qT_d = nc.dram_tensor("qT", [HID, NPC], f32, kind="ExternalInput").ap()
    hT_d = nc.dram_tensor("hT", [HID, NPC], bf, kind="ExternalInput").ap()
    o1T_d = nc.dram_tensor("o1T", [HID, NPC], bf, kind="ExternalInput").ap()
    dinv_nm_d = nc.dram_tensor("dinv_nm", [128, NSB], f32, kind="ExternalInput").ap()
    sd_row_d = nc.dram_tensor("sd_row", [1, NPC], bf, kind="ExternalInput").ap()
    bv2r_d = nc.dram_tensor("bv2r", [1, HID], bf, kind="ExternalInput").ap()
    wk3_d = nc.dram_tensor("wk3", [HID, HID], bf, kind="ExternalInput").ap()
    wv3_d = nc.dram_tensor("wv3", [HID, HID], bf, kind="ExternalInput").ap()
    wq3_d = nc.dram_tensor("wq3", [HID, HID], bf, kind="ExternalInput").ap()
    bq3_d = nc.dram_tensor("bq3", [HID, 1], f32, kind="ExternalInput").ap()
    kd0_d = nc.dram_tensor("kd0", [HID, NPC], bf, kind="ExternalOutput").ap()
    kd1_d = nc.dram_tensor("kd1", [HID, NPC], bf, kind="ExternalOutput").ap()
    vd0_d = nc.dram_tensor("vd0", [HID, NPC], bf, kind="ExternalOutput").ap()
    vd1_d = nc.dram_tensor("vd1", [HID, NPC], bf, kind="ExternalOutput").ap()
    v2s_d = nc.dram_tensor("v2s", [NPC, HID], bf, kind="ExternalOutput").ap()
    q3T_d = nc.dram_tensor("q3T", [HID, NPC], f32, kind="ExternalOutput").ap()

    with tile.TileContext(nc) as tc, ExitStack() as ctx:
        ident_f, ident_b = _consts(nc, tc, ctx)
        ed_pool = ctx.enter_context(tc.tile_pool(name="ed", bufs=3))
        pref = []
        for (t0, gw, segs) in plan[:1]:
            edp = ed_pool.tile([128, CH, roww], bf, tag="ed")
            nc.gpsimd.dma_start(edp[:, :gw], ed_d[:, t0 : t0 + gw, :])
            pref.append(edp)
        meta_pool = ctx.enter_context(tc.tile_pool(name="meta", bufs=1))
        dj8_t = meta_pool.tile([128, tt, 8], bf, tag="dj8")
        nc.sync.dma_start(dj8_t[:], dj8_d[:])
        qrows = _qrows_from_cols(nc, tc, ctx, qT_d, ident_f)
        wpool = ctx.enter_context(tc.tile_pool(name="w", bufs=1))
        hpool = ctx.enter_context(tc.tile_pool(name="h", bufs=1))
        wk3t = _load_w(nc, wpool, wk3_d, HID, HID, "wk3t", bf)
        wv3t = _load_w(nc, wpool, wv3_d, HID, HID, "wv3t", bf)
        wq3t = _load_w(nc, wpool, wq3_d, HID, HID, "wq3t", bf)
        bq3t = _load_w(nc, wpool, bq3_d, HID, 1, "bq3t")
        dinv_nm = _load_w(nc, wpool, dinv_nm_d, 128, NSB, "dinv")
        sd_row = _load_w(nc, wpool, sd_row_d, 1, NPC, "sdrow", bf)
        bv2r = _load_w(nc, wpool, bv2r_d, 1, HID, "bv2r", bf)
        hT = hpool.tile([HID, NPC], bf, tag="hT")
        nc.sync.dma_start(hT[:], hT_d[:])
        o1T = hpool.tile([HID, NPC], bf, tag="o1T")
        nc.sync.dma_start(o1T[:], o1T_d[:])
        o2T = hpool.tile([HID, NPC], bf, tag="o2T")
        dk_pool = ctx.enter_context(tc.tile_pool(name="dk", bufs=2))
        sc_pool = ctx.enter_context(tc.tile_pool(name="sc", bufs=2))
        msg_pool = ctx.enter_context(tc.tile_pool(name="msg", bufs=2))
        st_pool = ctx.enter_context(tc.tile_pool(name="st", bufs=2))
        tab_pool = ctx.enter_context(tc.tile_pool(name="tab", bufs=1))
        psum_seg = ctx.enter_context(tc.tile_pool(name="pseg", bufs=2, space="PSUM"))
        psum_t = ctx.enter_context(tc.tile_pool(name="pt", bufs=1, space="PSUM"))
        psum_m = ctx.enter_context(tc.tile_pool(name="pm", bufs=2, space="PSUM"))

        # h/o1-side layer-3 tables (independent of o2; overlaps edge ramp)
        khT3 = hpool.tile([HID, NPC], bf, tag="khT3")
        _proj_sbuf(nc, wk3t, hT, khT3, psum_m)
        ko1T3 = hpool.tile([HID, NPC], bf, tag="ko1T3")
        _proj_sbuf(nc, wk3t, o1T, ko1T3, psum_m)
        vhT3 = hpool.tile([HID, NPC], bf, tag="vhT3")
        _proj_sbuf(nc, wv3t, hT, vhT3, psum_m)
        vo1T3 = hpool.tile([HID, NPC], bf, tag="vo13")
        _proj_sbuf(nc, wv3t, o1T, vo1T3, psum_m)

        def stageA(ed_t, segs, t0, gw):
            # z = q . kd  (d-major tree), a0 = sigmoid(z)
            dk = dk_pool.tile([128, CH, HID], bf, tag="dk")
            for (b, off, w, fi, la) in segs:
                nc.vector.tensor_tensor(
                    out=dk[:, off : off + w], in0=ed_t[:, off : off + w, 0:HID],
                    in1=qrows[:, b : b + 1, :].to_broadcast([128, w, HID]),
                    op=AT.mult,
                )
            dk5 = dk[:, :gw].rearrange("p c (d h) -> p c d h", d=8)
            r4 = sc_pool.tile([128, CH, 4, 8], bf, tag="r4")
            nc.vector.tensor_tensor(
                out=r4[:, :gw], in0=dk5[:, :, 0:4], in1=dk5[:, :, 4:8], op=AT.add
            )
            r2 = sc_pool.tile([128, CH, 2, 8], bf, tag="r2")
            nc.vector.tensor_tensor(
                out=r2[:, :gw], in0=r4[:, :gw, 0:2], in1=r4[:, :gw, 2:4], op=AT.add
            )
            z = sc_pool.tile([128, CH, 8], bf, tag="z")
            nc.vector.tensor_tensor(
                out=z[:, :gw], in0=r2[:, :gw, 0], in1=r2[:, :gw, 1], op=AT.add
            )
            a0 = sc_pool.tile([128, CH, 8], bf, tag="a0")
            nc.scalar.activation(a0[:, :gw], z[:, :gw], Sig)
            return (ed_t, a0, t0, gw)

        def stageB(aobj):
            (ed_t, a0, t0, gw) = aobj
            a0s = sc_pool.tile([128, CH, 8], bf, tag="a0s")
            nc.vector.tensor_tensor(
                out=a0s[:, :gw], in0=a0[:, :gw], in1=dj8_t[:, t0 : t0 + gw],
                op=AT.mult,
            )
            dv = ed_t[:, :gw, HID : 2 * HID].rearrange("p c (d h) -> p c d h", d=8)
            mdv = msg_pool.tile([128, CH, 8, 8], bf, tag="mdv")
            nc.vector.tensor_tensor(
                out=mdv[:, :gw], in0=dv,
                in1=a0s[:, :gw, None, :].to_broadcast([128, gw, 8, 8]),
                op=AT.mult,
            )
            # msum = a0*dinvj*dv + v1s  (v1s already dinv_j-scaled)
            msum = msg_pool.tile([128, CH, HID], bf, tag="msum")
            nc.vector.tensor_tensor(
                out=msum[:, :gw].rearrange("p c (d h) -> p c d h", d=8),
                in0=mdv[:, :gw],
                in1=ed_t[:, :gw, 2 * HID : 3 * HID].rearrange(
                    "p c (d h) -> p c d h", d=8
                ),
                op=AT.add,
            )
            return (msum, gw)

        def group_tables(b):
            # tables for sbs [b, b+3] in one 512-col batch
            j0 = b * SBT
            wg = min(4 * SBT, NPC - j0)
            o2g = o2T[:, j0 : j0 + wg]
            ko2 = psum_m.tile([HID, 512], f32, tag="mt")
            nc.tensor.matmul(out=ko2[:, :wg], lhsT=wk3t[:], rhs=o2g, start=True, stop=True)
            kd0c = tab_pool.tile([HID, 512], bf, tag="kd0c")
            nc.vector.tensor_tensor(out=kd0c[:, :wg], in0=khT3[:, j0 : j0 + wg], in1=ko2[:, :wg], op=AT.subtract)
            nc.sync.dma_start(kd0_d[:, j0 : j0 + wg], kd0c[:, :wg])
            kd1c = tab_pool.tile([HID, 512], bf, tag="kd1c")
            nc.vector.tensor_tensor(out=kd1c[:, :wg], in0=ko1T3[:, j0 : j0 + wg], in1=ko2[:, :wg], op=AT.subtract)
            nc.sync.dma_start(kd1_d[:, j0 : j0 + wg], kd1c[:, :wg])
            vo2 = psum_m.tile([HID, 512], f32, tag="mt")
            nc.tensor.matmul(out=vo2[:, :wg], lhsT=wv3t[:], rhs=o2g, start=True, stop=True)
            vd0c = tab_pool.tile([HID, 512], bf, tag="vd0c")
            nc.vector.tensor_tensor(out=vd0c[:, :wg], in0=vhT3[:, j0 : j0 + wg], in1=vo2[:, :wg], op=AT.subtract)
            nc.sync.dma_start(vd0_d[:, j0 : j0 + wg], vd0c[:, :wg])
            vd1c = tab_pool.tile([HID, 512], bf, tag="vd1c")
            nc.vector.tensor_tensor(out=vd1c[:, :wg], in0=vo1T3[:, j0 : j0 + wg], in1=vo2[:, :wg], op=AT.subtract)
            nc.sync.dma_start(vd1_d[:, j0 : j0 + wg], vd1c[:, :wg])
            # v2s rows: dinv_j-scaled transposes of v2 = wv3.T o2
            v2g = tab_pool.tile([HID, 512], bf, tag="v2g")
            nc.scalar.copy(v2g[:, :wg], vo2[:, :wg])
            for k in range((wg + SBT - 1) // SBT):
                wk_ = min(SBT, wg - k * SBT)
                pv = psum_t.tile([128, HID], bf, tag="pv")
                nc.tensor.transpose(
                    out=pv[:wk_], in_=v2g[:, k * SBT : k * SBT + wk_],
                    identity=ident_b[:HID, :HID],
                )
                v2s = tab_pool.tile([128, HID], bf, tag="v2s")
                nc.scalar.activation(
                    v2s[:wk_], pv[:wk_], Ident, scale=dinv_nm[:wk_, b + k : b + k + 1]
                )
                nc.sync.dma_start(v2s_d[j0 + k * SBT : j0 + k * SBT + wk_, :], v2s[:wk_])
            # q3 columns
            q3 = psum_m.tile([HID, 512], f32, tag="mt")
            nc.tensor.matmul(out=q3[:, :wg], lhsT=wq3t[:], rhs=o2g, start=True, stop=True)
            q3s = tab_pool.tile([HID, 512], f32, tag="q3s")
            nc.scalar.activation(q3s[:, :wg], q3[:, :wg], Ident, bias=bq3t[:])
            nc.sync.dma_start(q3T_d[:, j0 : j0 + wg], q3s[:, :wg])

        def out_cb(b, psT):
            j0, w = _sb_w(b)
            st = st_pool.tile([128, HID], bf, tag="st")
            nc.scalar.activation(st[:w], psT[:w], Relu, scale=dinv_nm[:w, b : b + 1])
            pt = psum_t.tile([HID, 128], bf, tag="pt")
            nc.tensor.transpose(out=pt[:, :w], in_=st[:w], identity=ident_b[:w, :w])
            o2c = o2T[:, j0 : j0 + w]
            nc.scalar.copy(o2c, pt[:, :w])
            if b % 4 == 0:
                group_tables(b)

        state = {"ps": None, "pend_cb": None}

        def finish(segs, bobj):
            (msum, gw) = bobj
            for (b, off, w, fi, la) in segs:
                if fi:
                    ps_new = psum_seg.tile([128, HID], f32, tag="ps")
                    state["ps"] = ps_new
                ps = state["ps"]
                for i in range(w):
                    nc.tensor.matmul(
                        out=ps[:], lhsT=ident_b[:], rhs=msum[:, off + i],
                        start=(fi and i == 0), stop=False,
                    )
                if la:
                    j0, w2 = _sb_w(b)
                    nc.tensor.matmul(
                        out=ps[:w2], lhsT=sd_row[:, j0 : j0 + w2], rhs=bv2r[:],
                        start=False, stop=True,
                    )
                    if state["pend_cb"] is not None:
                        out_cb(*state["pend_cb"])
                    state["pend_cb"] = (b, ps)

        pendA = None
        for ci, (t0, gw, segs) in enumerate(plan):
            if ci < 1:
                ed_t = pref[ci]
            else:
                ed_t = ed_pool.tile([128, CH, roww], bf, tag="ed")
                nc.gpsimd.dma_start(ed_t[:, :gw], ed_d[:, t0 : t0 + gw, :])
            aobj = stageA(ed_t, segs, t0, gw)
            if pendA is not None:
                (psegs, paobj) = pendA
                finish(psegs, stageB(paobj))
            pendA = (segs, aobj)
        (psegs, paobj) = pendA
        finish(psegs, stageB(paobj))
        if state["pend_cb"] is not None:
            out_cb(*state["pend_cb"])
    _split_multi_waits(nc)
    return nc


# ---------------------------------------------------------------- launch D (layer 3 + head)
def _build_launch_D(tt, tps):
    import concourse.tile as tile
    from concourse import mybir
    from contextlib import ExitStack

    f32 = mybir.dt.float32
    bf = mybir.dt.bfloat16
    AT = mybir.AluOpType
    Exp = mybir.ActivationFunctionType.Exp
    Ln = mybir.ActivationFunctionType.Ln
    nc = _mk_nc()
    roww = 5 * HID  # 320: [kd0 | kd1 | vd0 | vd1 | v2s]
    CH = 40
    plan = _plan_super(tps, CH)

    ed_d = nc.dram_tensor("ed", [128, tt, roww], bf, kind="ExternalInput").ap()
    dj8_d = nc.dram_tensor("dj8", [128, tt, 8], bf, kind="ExternalInput").ap()
    qT_d = nc.dram_tensor("qT", [HID, NPC], f32, kind="ExternalInput").ap()
    dinv_nm_d = nc.dram_tensor("dinv_nm", [128, NSB], f32, kind="ExternalInput").ap()
    sd_row_d = nc.dram_tensor("sd_row", [1, NPC], bf, kind="ExternalInput").ap()
    bv3r_d = nc.dram_tensor("bv3r", [1, HID], bf, kind="ExternalInput").ap()
    w2_d = nc.dram_tensor("w2", [HID, OUT_C], bf, kind="ExternalInput").ap()
    b2bc_d = nc.dram_tensor("b2bc", [128, OUT_C], f32, kind="ExternalInput").ap()
    y_d = nc.dram_tensor("y", [NPC, OUT_C], f32, kind="ExternalOutput").ap()

    with tile.TileContext(nc) as tc, ExitStack() as ctx:
        ident_f, ident_b = _consts(nc, tc, ctx)
        ed_pool = ctx.enter_context(tc.tile_pool(name="ed", bufs=3))
        pref = []
        for (t0, gw, segs) in plan[:1]:
            edp = ed_pool.tile([128, CH, roww], bf, tag="ed")
            nc.gpsimd.dma_start(edp[:, :gw], ed_d[:, t0 : t0 + gw, :])
            pref.append(edp)
        meta_pool = ctx.enter_context(tc.tile_pool(name="meta", bufs=1))
        dj8_t = meta_pool.tile([128, tt, 8], bf, tag="dj8")
        nc.sync.dma_start(dj8_t[:], dj8_d[:])
        qrows = _qrows_from_cols(nc, tc, ctx, qT_d, ident_f)
        wpool = ctx.enter_context(tc.tile_pool(name="w", bufs=1))
        w2t = _load_w(nc, wpool, w2_d, HID, OUT_C, "w2t", bf)
        b2t = _load_w(nc, wpool, b2bc_d, 128, OUT_C, "b2t")
        dinv_nm = _load_w(nc, wpool, dinv_nm_d, 128, NSB, "dinv")
        sd_row = _load_w(nc, wpool, sd_row_d, 1, NPC, "sdrow", bf)
        bv3r = _load_w(nc, wpool, bv3r_d, 1, HID, "bv3r", bf)
        dk_pool = ctx.enter_context(tc.tile_pool(name="dk", bufs=2))
        sc_pool = ctx.enter_context(tc.tile_pool(name="sc", bufs=2))
        msg_pool = ctx.enter_context(tc.tile_pool(name="msg", bufs=2))
        st_pool = ctx.enter_context(tc.tile_pool(name="st", bufs=2))
        sm_pool = ctx.enter_context(tc.tile_pool(name="sm", bufs=2))
        psum_seg = ctx.enter_context(tc.tile_pool(name="pseg", bufs=2, space="PSUM"))
        psum_t = ctx.enter_context(tc.tile_pool(name="pt", bufs=2, space="PSUM"))
        psum_lg = ctx.enter_context(tc.tile_pool(name="plg", bufs=2, space="PSUM"))

        def stageA(ed_t, segs, t0, gw):
            # z_t = q . kd_t (d-major tree), ee = exp(z),
            # rz = 1/(1+e0+e1) = exp(-ln(1 + e0 + e1))  (Exp/Ln only)
            dk = dk_pool.tile([128, CH, 2, HID], bf, tag="dk")
            for (b, off, w, fi, la) in segs:
                nc.vector.tensor_tensor(
                    out=dk[:, off : off + w],
                    in0=ed_t[:, off : off + w, 0 : 2 * HID].rearrange(
                        "p c (t d) -> p c t d", t=2
                    ),
                    in1=qrows[:, b : b + 1, None, :].to_broadcast([128, w, 2, HID]),
                    op=AT.mult,
                )
            dk5 = dk[:, :gw].rearrange("p c t (d h) -> p c t d h", d=8)
            r4 = sc_pool.tile([128, CH, 2, 4, 8], bf, tag="r4")
            nc.vector.tensor_tensor(
                out=r4[:, :gw], in0=dk5[:, :, :, 0:4], in1=dk5[:, :, :, 4:8], op=AT.add
            )
            r2 = sc_pool.tile([128, CH, 2, 2, 8], bf, tag="r2")
            nc.vector.tensor_tensor(
                out=r2[:, :gw], in0=r4[:, :gw, :, 0:2], in1=r4[:, :gw, :, 2:4], op=AT.add
            )
            sc = sc_pool.tile([128, CH, 2, 8], bf, tag="sc")
            nc.vector.tensor_tensor(
                out=sc[:, :gw], in0=r2[:, :gw, :, 0], in1=r2[:, :gw, :, 1], op=AT.add
            )
            ee = sc_pool.tile([128, CH, 2, 8], bf, tag="ee")
            nc.scalar.activation(ee[:, :gw], sc[:, :gw], Exp)
            dd = sc_pool.tile([128, CH, 8], bf, tag="dd")
            nc.vector.tensor_tensor(
                out=dd[:, :gw], in0=ee[:, :gw, 0], in1=ee[:, :gw, 1], op=AT.add
            )
            lse = sc_pool.tile([128, CH, 8], f32, tag="lse")
            nc.scalar.activation(lse[:, :gw], dd[:, :gw], Ln, bias=1.0)
            rz = sc_pool.tile([128, CH, 8], bf, tag="rz")
            nc.scalar.activation(rz[:, :gw], lse[:, :gw], Exp, scale=-1.0)
            return (ed_t, ee, rz, t0, gw)

        def stageB(mobj):
            (ed_t, ee, rz, t0, gw) = mobj
            rzs = sc_pool.tile([128, CH, 8], bf, tag="rzs")
            nc.vector.tensor_tensor(
                out=rzs[:, :gw], in0=rz[:, :gw], in1=dj8_t[:, t0 : t0 + gw],
                op=AT.mult,
            )
            aa = sc_pool.tile([128, CH, 2, 8], bf, tag="aa")
            nc.vector.tensor_tensor(
                out=aa[:, :gw], in0=ee[:, :gw],
                in1=rzs[:, :gw, None, :].to_broadcast([128, gw, 2, 8]),
                op=AT.mult,
            )
            ve = ed_t[:, :gw, 2 * HID : 4 * HID].rearrange(
                "p c (t d h) -> p c t d h", t=2, d=8
            )
            mm = msg_pool.tile([128, CH, 2, 8, 8], bf, tag="mm")
            nc.vector.tensor_tensor(
                out=mm[:, :gw], in0=ve,
                in1=aa[:, :gw, :, None, :].to_broadcast([128, gw, 2, 8, 8]),
                op=AT.mult,
            )
            m01 = msg_pool.tile([128, CH, 8, 8], bf, tag="m01")
            nc.vector.tensor_tensor(
                out=m01[:, :gw], in0=mm[:, :gw, 0], in1=mm[:, :gw, 1], op=AT.add
            )
            # msum = m01 + v2s  (v2s carries dinv_j; its attn weight sums to 1)
            msum = msg_pool.tile([128, CH, HID], bf, tag="msum")
            nc.vector.tensor_tensor(
                out=msum[:, :gw].rearrange("p c (d h) -> p c d h", d=8),
                in0=m01[:, :gw],
                in1=ed_t[:, :gw, 4 * HID : 5 * HID].rearrange(
                    "p c (d h) -> p c d h", d=8
                ),
                op=AT.add,
            )
            return (msum, gw)

        def out_cb(b, psT):
            j0, w = _sb_w(b)
            # relu + dinv_i scale on the DVE (keeps ScalarE's ACT table small)
            st = st_pool.tile([128, HID], bf, tag="st")
            nc.vector.tensor_scalar(
                out=st[:w], in0=psT[:w], scalar1=dinv_nm[:w, b : b + 1],
                scalar2=0.0, op0=AT.mult, op1=AT.max,
            )
            pt = psum_t.tile([HID, 128], bf, tag="pt")
            nc.tensor.transpose(out=pt[:, :w], in_=st[:w], identity=ident_b[:w, :w])
            o3T = st_pool.tile([HID, 128], bf, tag="o3T")
            nc.scalar.copy(o3T[:, :w], pt[:, :w])
            lg = psum_lg.tile([128, OUT_C], f32, tag="lg")
            nc.tensor.matmul(out=lg[:w], lhsT=o3T[:, :w], rhs=w2t[:], start=True, stop=True)
            logits = sm_pool.tile([128, OUT_C], f32, tag="logits")
            nc.vector.tensor_tensor(out=logits[:w], in0=lg[:w], in1=b2t[:w], op=AT.add)
            nlmax = sm_pool.tile([128, 1], f32, tag="nlmax")
            nc.vector.tensor_reduce(
                out=nlmax[:w], in_=logits[:w], axis=mybir.AxisListType.X,
                op=AT.max, negate=True,
            )
            eb = sm_pool.tile([128, OUT_C], f32, tag="eb")
            esum = sm_pool.tile([128, 1], f32, tag="esum")
            nc.scalar.activation(
                eb[:w], logits[:w], Exp, bias=nlmax[:w], accum_out=esum[:w]
            )
            lse2 = sm_pool.tile([128, 1], f32, tag="lse2")
            nc.scalar.activation(lse2[:w], esum[:w], Ln)
            off = sm_pool.tile([128, 1], f32, tag="off")
            nc.vector.tensor_tensor(out=off[:w], in0=lse2[:w], in1=nlmax[:w], op=AT.subtract)
            yy = sm_pool.tile([128, OUT_C], f32, tag="yy")
            nc.vector.tensor_tensor(
                out=yy[:w], in0=logits[:w],
                in1=off[:w].to_broadcast([w, OUT_C]), op=AT.subtract,
            )
            nc.sync.dma_start(y_d[j0 : j0 + w, :], yy[:w])

        state = {"ps": None, "pend_cb": None}

        def finish(segs, bobj):
            (msum, gw) = bobj
            for (b, off, w, fi, la) in segs:
                if fi:
                    ps_new = psum_seg.tile([128, HID], f32, tag="ps")
                    state["ps"] = ps_new
                ps = state["ps"]
                for i in range(w):
                    nc.tensor.matmul(
                        out=ps[:], lhsT=ident_b[:], rhs=msum[:, off + i],
                        start=(fi and i == 0), stop=False,
                    )
                if la:
                    j0, w2 = _sb_w(b)
                    nc.tensor.matmul(
                        out=ps[:w2], lhsT=sd_row[:, j0 : j0 + w2], rhs=bv3r[:],
                        start=False, stop=True,
                    )
                    if state["pend_cb"] is not None:
                        out_cb(*state["pend_cb"])
                    state["pend_cb"] = (b, ps)

        pendA = None
        for ci, (t0, gw, segs) in enumerate(plan):
            if ci < 1:
                ed_t = pref[ci]
            else:
                ed_t = ed_pool.tile([128, CH, roww], bf, tag="ed")
                nc.gpsimd.dma_start(ed_t[:, :gw], ed_d[:, t0 : t0 + gw, :])
            aobj = stageA(ed_t, segs, t0, gw)
            if pendA is not None:
                (psegs, paobj) = pendA
                finish(psegs, stageB(paobj))
            pendA = (segs, aobj)
        (psegs, paobj) = pendA
        finish(psegs, stageB(paobj))
        if state["pend_cb"] is not None:
            out_cb(*state["pend_cb"])
    _split_multi_waits(nc)
    return nc


# ---------------------------------------------------------------- host gather
def _u16(a):
    return a.view(np.uint16)


def _gather_cat(tabs, eidx):
    """[128, TT, sum(w)] bf16: rows gathered by global src id from each
    table [N+1, w] (row N is the zero sentinel), concatenated."""
    tt = eidx.shape[1]
    ws = [t.shape[1] for t in tabs]
    out = np.empty((128, tt, sum(ws)), dtype=np.uint16)
    o = 0
    for t, w in zip(tabs, ws):
        out[:, :, o : o + w] = _u16(t)[eidx]
        o += w
    return out.view(BF16)


def _scatter_cols(cols_list, ids):
    """tab[global_id] = cols.T for each core; sentinel zero row."""
    w = cols_list[0].shape[0]
    tab = np.zeros((N + 1, w), dtype=BF16)
    for c in range(NCORES):
        tab[ids[c]] = cols_list[c].T
    return tab


def _scatter_rows(rows_list, ids):
    w = rows_list[0].shape[1]
    tab = np.zeros((N + 1, w), dtype=BF16)
    for c in range(NCORES):
        tab[ids[c]] = rows_list[c]
    return tab


# ---------------------------------------------------------------- driver
def kernel(x, edge_index, lin1_w, lin1_b, wq, bq, wk, bk, wv, bv, lin2_w, lin2_b):
    _install_fixups()
    from concourse.bass_utils import run_bass_kernel_spmd

    x = np.asarray(x, dtype=np.float32)
    lin1_w = np.asarray(lin1_w, np.float32)
    lin1_b = np.asarray(lin1_b, np.float32)
    wq = np.asarray(wq, np.float32)
    bq = np.asarray(bq, np.float32)
    wk = np.asarray(wk, np.float32)
    wv = np.asarray(wv, np.float32)
    bv = np.asarray(bv, np.float32)
    lin2_w = np.asarray(lin2_w, np.float32)
    lin2_b = np.asarray(lin2_b, np.float32)
    isd = np.float32(1.0 / np.sqrt(DH))

    metas, tps, tt, chunks, s_all, dinv, ids = _preprocess(np.asarray(edge_index))

    key = ("progs", tps, tt)
    if key not in _CACHE:
        _CACHE[key] = (
            _build_launch_A(),
            _build_launch_B(tt, chunks),
            _build_launch_C(tt, tps),
            _build_launch_D(tt, tps),
        )
    ncA, ncB, ncC, ncD = _CACHE[key]
    cores = list(range(NCORES))

    def conj(W):  # d-major conjugation
        return W[PRM][:, PRM]

    # per-core metadata columns
    dinv_nm = []
    s_rows = []
    sd_rows = []
    for c in cores:
        dv = dinv[ids[c]]
        dm = np.ones((128, NSB), np.float32)
        for b in range(NSB):
            j0, w = b * SBT, min(SBT, NPC - b * SBT)
            dm[:w, b] = dv[j0 : j0 + w]
        dinv_nm.append(dm)
        s_rows.append(s_all[ids[c]][None, :].astype(BF16))
        sd_rows.append((s_all[ids[c]] / dv)[None, :].astype(BF16))

    # ---- launch A: h = relu(x @ W1 + b1) (PRM basis), columnar bf16
    xT = np.ascontiguousarray(x.T).astype(BF16)
    w1_bf = lin1_w[:, PRM].astype(BF16)
    b1_prm = lin1_b[PRM][:, None]
    in_maps = [
        dict(
            xT=np.ascontiguousarray(xT[:, ids[c]]),
            w1=w1_bf,
            b1=b1_prm,
        )
        for c in cores
    ]
    resA = run_bass_kernel_spmd(ncA, in_maps, cores)
    hT = [np.asarray(resA.results[c]["hT_out"]) for c in cores]
    h_tab = _scatter_cols(hT, ids)

    # ---- launch B: layer 1 (attn == identity) + kd/dv/v1s/q2 tables
    in_maps = [
        dict(
            ed=_gather_cat([h_tab], metas[c]["eidx"]),
            dj8=metas[c]["dj8"],
            hT=hT[c],
            dinv_nm=dinv_nm[c],
            s_row=s_rows[c],
            bv0r=bv[0][PRM][None, :].astype(BF16),
            wv0=conj(wv[0]).astype(BF16),
            wk2=conj(wk[1]).astype(BF16),
            wv2=conj(wv[1]).astype(BF16),
            wq2=(conj(wq[1]) * isd).astype(BF16),
            bq2=(bq[1][PRM] * isd)[:, None],
        )
        for c in cores
    ]
    resB = run_bass_kernel_spmd(ncB, in_maps, cores)
    o1T = [np.asarray(resB.results[c]["o1T"]) for c in cores]
    q2T = [np.asarray(resB.results[c]["q2T"]) for c in cores]
    kd_tab = _scatter_cols([np.asarray(resB.results[c]["kd"]) for c in cores], ids)
    dv_tab = _scatter_cols([np.asarray(resB.results[c]["dv"]) for c in cores], ids)
    v1s_tab = _scatter_rows([np.asarray(resB.results[c]["v1s"]) for c in cores], ids)

    # ---- launch C: layer 2 + kd0/kd1/vd0/vd1/v2s/q3 tables
    in_maps = [
        dict(
            ed=_gather_cat([kd_tab, dv_tab, v1s_tab], metas[c]["eidx"]),
            dj8=metas[c]["dj8"],
            qT=q2T[c],
            hT=hT[c],
            o1T=o1T[c],
            dinv_nm=dinv_nm[c],
            sd_row=sd_rows[c],
            bv2r=bv[1][PRM][None, :].astype(BF16),
            wk3=conj(wk[2]).astype(BF16),
            wv3=conj(wv[2]).astype(BF16),
            wq3=(conj(wq[2]) * isd).astype(BF16),
            bq3=(bq[2][PRM] * isd)[:, None],
        )
        for c in cores
    ]
    resC = run_bass_kernel_spmd(ncC, in_maps, cores)
    q3T = [np.asarray(resC.results[c]["q3T"]) for c in cores]
    kd0_tab = _scatter_cols([np.asarray(resC.results[c]["kd0"]) for c in cores], ids)
    kd1_tab = _scatter_cols([np.asarray(resC.results[c]["kd1"]) for c in cores], ids)
    vd0_tab = _scatter_cols([np.asarray(resC.results[c]["vd0"]) for c in cores], ids)
    vd1_tab = _scatter_cols([np.asarray(resC.results[c]["vd1"]) for c in cores], ids)
    v2s_tab = _scatter_rows([np.asarray(resC.results[c]["v2s"]) for c in cores], ids)

    # ---- launch D: layer 3 + classifier head + log_softmax
    b2bc = np.ascontiguousarray(np.broadcast_to(lin2_b[None, :], (128, OUT_C)))
    in_maps = [
        dict(
            ed=_gather_cat(
                [kd0_tab, kd1_tab, vd0_tab, vd1_tab, v2s_tab], metas[c]["eidx"]
            ),
            dj8=metas[c]["dj8"],
            qT=q3T[c],
            dinv_nm=dinv_nm[c],
            sd_row=sd_rows[c],
            bv3r=bv[2][PRM][None, :].astype(BF16),
            w2=lin2_w[PRM, :].astype(BF16),
            b2bc=b2bc,
        )
        for c in cores
    ]
    resD = run_bass_kernel_spmd(ncD, in_maps, cores)
    y = np.empty((N, OUT_C), dtype=np.float32)
    for c in cores:
        y[ids[c]] = np.asarray(resD.results[c]["y"], dtype=np.float32)
    return y


# revision 31
# speedup vs baseline: 1.0881x; 1.0881x over previous
"""Trainium2 Bass kernel for 3-layer GNN message passing with per-edge
multi-head attention over node history, distributed over 8 NeuronCores.

Sharding: nodes are relabeled by descending degree and dealt into
(superblock, core, slot) so that each 128-edge tile maps partition p <->
target slot p ("identity segment" scheme). Per-edge k/v history rows are
assembled on the host between launches (pure indexing) and streamed as
dense bf16.

Key structure (all FLOPs on device):
- segment-sum accumulates PSUM[slot, feat] with identity-stationary
  matmuls; per-edge messages are pre-combined on the DVE so each 128-edge
  tile costs one matmul (layer 1 tree-reduces whole chunks to one matmul).
- gcn_norm is separable (nrm_e = dinv_src * dinv_tgt): dinv_src rides in
  the dj8 stream / prescaled v-tables, dinv_tgt in the output activation
  scale, and the v-projection bias is restored after aggregation with a
  rank-1 (s/dinv x bv) matmul, using that attention weights sum to one.
- attention is in difference form: layer 2 needs only
  a0 = sigmoid(q . (k0-k1)); layer 3 uses e_t = exp(q . (k_t - k2)),
  1/Z = sigmoid(-ln(e0+e1)). k-projection biases cancel, streams are
  bias-free.
- next-layer tables are produced per-superblock inside the aggregation
  callback, so table building overlaps the edge stream instead of
  trailing it.
- everything is conjugated by the d-major head permutation so DVE ops are
  unit-stride bf16 (2x mode); each launch keeps <= 4 distinct ScalarE
  activation functions to avoid ACT_TABLE_LOAD thrash.
4 launches: proj, layer1, layer2, layer3+head.
"""

import sys
import types

import numpy as np
import ml_dtypes

sys.path.insert(0, "/opt/trn_rl_repo")

BF16 = ml_dtypes.bfloat16

# ---------------------------------------------------------------- fixups
_HOOK = [None]


def _install_fixups():
    if "antenv.axon_hooks" not in sys.modules:
        mod = types.ModuleType("antenv.axon_hooks")
        mod.set_axon_ntff_profile_hook = lambda h: _HOOK.__setitem__(0, h)
        mod.get_axon_ntff_profile_hook = lambda: _HOOK[0]
        sys.modules["antenv.axon_hooks"] = mod
        try:
            from trn_agent_boot.trn_boot import _ntff_profile_via_ctypes

            _HOOK[0] = _ntff_profile_via_ctypes("/opt/axon/libaxon_pjrt.so")
        except Exception:
            pass

    import concourse.tile as tile
    from concourse.vector_clock import ScopedClock
    import bass_rust

    if getattr(tile.TileContext, "_drain_split_installed", False):
        return

    def _drain_and_barrier(self, tick_clock, wait_clock):
        nc = self.nc
        drain_inst = nc.sync.drain()
        wait_clock.add_sem_waits(
            drain_inst.ins, ScopedClock({None: tick_clock.global_clock})
        )
        si = drain_inst.ins.sync_info
        waits = list(si.on_wait or []) if si is not None else []
        if len(waits) > 1:
            si.on_wait = waits[:1]
            for i in range(1, len(waits)):
                d2 = nc.sync.drain()
                d2.ins.sync_info = bass_rust.SyncInfo(
                    on_wait=waits[i : i + 1], on_update=[]
                )
        nc.all_engine_barrier()
        assert self.sems is not None
        popped = nc._tile_sem_poison_stack.pop()
        assert popped is self._sem_poison
        nc.clear_and_free_semaphores(list(self.sems.allocated().values()))
        nc.all_engine_barrier()

    tile.TileContext._drain_and_barrier = _drain_and_barrier
    tile.TileContext._drain_split_installed = True


# ---------------------------------------------------------------- constants
N = 20000
E = 320000
IN_C = 256
HID = 64
OUT_C = 64
HEADS = 8
DH = 8
NCORES = 8
NPC = N // NCORES  # 2500
SBT = 128  # target slots per superblock
NSB = (NPC + SBT - 1) // SBT  # 20 (last has 68 targets)
LASTW = NPC - (NSB - 1) * SBT  # 68
MAXG = 32  # max tiles per streamed chunk

# d-major permutation of the 64 features (8 heads x 8 dims), an involution
PRM = np.arange(HID).reshape(HEADS, DH).T.reshape(-1)

_CACHE = {}


# ---------------------------------------------------------------- host prep
def _preprocess(edge_index):
    row = np.asarray(edge_index[0], dtype=np.int64)
    col = np.asarray(edge_index[1], dtype=np.int64)
    loop = np.arange(N, dtype=np.int64)
    row_all = np.concatenate([row, loop])
    col_all = np.concatenate([col, loop])
    deg = np.bincount(col_all, minlength=N).astype(np.int64)
    dinv = (1.0 / np.sqrt(np.maximum(deg, 1))).astype(np.float32)
    nrm = (dinv[row_all] * dinv[col_all]).astype(np.float32)
    s_all = np.bincount(col_all, weights=nrm.astype(np.float64), minlength=N)
    s_all = s_all.astype(np.float32)

    # degree-sorted relabeling: rank r -> (superblock b, core c, slot p)
    order = np.argsort(-deg, kind="stable")  # global ids by desc degree
    b_of = np.empty(N, np.int64)
    c_of = np.empty(N, np.int64)
    p_of = np.empty(N, np.int64)
    ranks = np.arange(N)
    full = (NSB - 1) * 1024  # ranks dealt in blocks of 8*128
    b_of[ranks < full] = ranks[ranks < full] // 1024
    c_of[ranks < full] = (ranks[ranks < full] % 1024) // SBT
    p_of[ranks < full] = ranks[ranks < full] % SBT
    tail = ranks >= full
    b_of[tail] = NSB - 1
    c_of[tail] = (ranks[tail] - full) // LASTW
    p_of[tail] = (ranks[tail] - full) % LASTW
    # per-node placement (indexed by global id)
    nb = np.empty(N, np.int64); nb[order] = b_of
    ncr = np.empty(N, np.int64); ncr[order] = c_of
    npp = np.empty(N, np.int64); npp[order] = p_of
    # ids[c][b*128+p] = global id owned by core c at local index
    ids = np.empty((NCORES, NPC), np.int64)
    loc = nb * SBT + npp  # local index within core
    ids[ncr, loc] = np.arange(N)

    # tiles per superblock = max degree within the superblock (desc sorted)
    tps = np.zeros(NSB, np.int64)
    for b in range(NSB):
        r0 = b * 1024 if b < NSB - 1 else full
        tps[b] = max(1, int(deg[order[r0]]))
    sb_start = np.zeros(NSB + 1, np.int64)
    sb_start[1:] = np.cumsum(tps)
    tt = int(sb_start[-1])

    # scatter edges: edge i (sorted by target) lands at
    # core c(t), row p(t), column sb_start[b(t)] + within-target-rank
    es = np.argsort(col_all, kind="stable")
    tgt = col_all[es]
    src = row_all[es]
    start_of = np.zeros(N + 1, np.int64)
    start_of[1:] = np.cumsum(np.bincount(tgt, minlength=N))
    rank_in_tgt = np.arange(len(tgt)) - start_of[tgt]
    dcol = sb_start[nb[tgt]] + rank_in_tgt
    drow = npp[tgt]
    dcore = ncr[tgt]

    dinv_ext = np.concatenate([dinv, np.zeros(1, np.float32)])
    metas = []
    for c in range(NCORES):
        m = dcore == c
        eidx = np.full((128, tt), N, np.int64)  # sentinel -> zero row
        eidx[drow[m], dcol[m]] = src[m]
        dj = dinv_ext[eidx]  # [128, tt] f32, 0 at padding
        dj8 = np.ascontiguousarray(
            np.broadcast_to(dj[:, :, None], (128, tt, 8))
        ).astype(BF16)
        metas.append(dict(eidx=eidx, dj8=dj8))

    # chunk plan: per sb, tiles split into chunks of <= MAXG; sbs processed
    # smallest-first so the pipeline ramps quickly
    chunks = []  # (sb, t0, gw, first, last)
    for b in range(NSB - 1, -1, -1):
        t0 = int(sb_start[b])
        left = int(tps[b])
        while left > 0:
            gw = min(MAXG, left)
            chunks.append(
                (b, t0, gw, t0 == int(sb_start[b]), left == gw)
            )
            t0 += gw
            left -= gw
    return metas, tuple(int(x) for x in tps), tt, chunks, s_all, dinv, ids


_WS_CTR = [0]


def _split_multi_waits(nc, maxw=1):
    """This container's walrus rejects instructions with more than one sync
    wait; hoist excess waits onto NoOps inserted before the instruction."""
    from concourse import mybir

    for f in nc.m.functions:
        for bb in f.blocks:
            insts = list(bb.instructions)
            out = []
            changed = False
            for inst in insts:
                si = inst.sync_info
                waits = list(si.on_wait) if (si is not None and si.on_wait) else []
                if len(waits) > maxw:
                    excess = waits[: len(waits) - maxw]
                    for j in range(0, len(excess), maxw):
                        _WS_CTR[0] += 1
                        out.append(
                            mybir.InstNoOp(
                                name=f"waitsplit_{_WS_CTR[0]}",
                                engine=inst.engine,
                                sync_info=mybir.SyncInfo(
                                    on_wait=excess[j : j + maxw], on_update=[]
                                ),
                                bass_nofuse=True,
                            )
                        )
                    si.on_wait = waits[len(waits) - maxw :]
                    changed = True
                out.append(inst)
            if changed:
                bb.instructions = out


def _mk_nc():
    import concourse.bass as bass

    return bass.Bass(num_devices=NCORES, debug=False, target_bir_lowering=False)


def _load_w(nc, pool, dram_ap, p, f, tag, dtype=None):
    from concourse import mybir

    t = pool.tile([p, f], dtype or mybir.dt.float32, tag=tag)
    nc.sync.dma_start(t[:], dram_ap[:])
    return t


def _proj_sbuf(nc, w_t, src, dst, psum_pool):
    """dst[64, NPC] (SBUF) = w.T @ src, bias-free, bf16, 500-col chunks."""
    from concourse import mybir

    f32 = mybir.dt.float32
    Ident = mybir.ActivationFunctionType.Identity
    NCH = 500
    for j0 in range(0, NPC, NCH):
        w = min(NCH, NPC - j0)
        ps = psum_pool.tile([HID, 512], f32, tag="mt")
        nc.tensor.matmul(
            out=ps[:, :w], lhsT=w_t[:], rhs=src[:, j0 : j0 + w],
            start=True, stop=True,
        )
        nc.scalar.activation(dst[:, j0 : j0 + w], ps[:, :w], Ident)


def _consts(nc, tc, ctx):
    from concourse import mybir
    from concourse.masks import make_identity

    cpool = ctx.enter_context(tc.tile_pool(name="const", bufs=1))
    ident_f = cpool.tile([128, 128], mybir.dt.float32, tag="idf")
    make_identity(nc, ident_f[:])
    ident_b = cpool.tile([128, 128], mybir.dt.bfloat16, tag="idb")
    nc.vector.tensor_copy(ident_b[:], ident_f[:])
    return ident_f, ident_b


def _qrows_from_cols(nc, tc, ctx, qT_d, ident_f):
    """Load q column-table [64, NPC] f32, transpose per superblock into
    qrows [128, NSB, 64] bf16 (row p = q of slot p; pad slots zeroed)."""
    from concourse import mybir

    f32 = mybir.dt.float32
    bf = mybir.dt.bfloat16
    qpool = ctx.enter_context(tc.tile_pool(name="q", bufs=1))
    qT = qpool.tile([HID, NPC], f32, tag="qT")
    nc.sync.dma_start(qT[:], qT_d[:])
    qrows = qpool.tile([128, NSB, HID], bf, tag="qrows")
    nc.vector.memset(qrows[:], 0.0)
    with tc.tile_pool(name="pqt", bufs=2, space="PSUM") as pst:
        for b in range(NSB):
            j0 = b * SBT
            w = min(SBT, NPC - j0)
            ps = pst.tile([128, HID], f32, tag="qtp")
            nc.tensor.transpose(
                out=ps[:w], in_=qT[:, j0 : j0 + w], identity=ident_f[:HID, :HID]
            )
            nc.scalar.copy(qrows[:w, b], ps[:w])
    return qrows



def _plan_super(tps, ch, first_ch=12):
    """Fixed contiguous column windows walked high->low; each knows its
    per-superblock segments (descending b). The first window is small so
    the edge pipeline starts computing before the bulk stream lands.
    Returns list of (c0, gw, segs) with segs = (b, off, w, first, last)."""
    sb_start = [0]
    for t in tps:
        sb_start.append(sb_start[-1] + t)
    tt = sb_start[-1]
    out = []
    c1 = tt
    while c1 > 0:
        c0 = max(0, c1 - (first_ch if not out else ch))
        segs = []
        for b in range(NSB - 1, -1, -1):
            lo = max(sb_start[b], c0)
            hi = min(sb_start[b + 1], c1)
            if hi <= lo:
                continue
            segs.append(
                (b, lo - c0, hi - lo, sb_start[b + 1] <= c1, sb_start[b] >= c0)
            )
        out.append((c0, c1 - c0, segs))
        c1 = c0
    return out


def _sb_w(b):
    j0 = b * SBT
    return j0, min(SBT, NPC - j0)


# ---------------------------------------------------------------- launch A
def _build_launch_A():
    import concourse.tile as tile
    from concourse import mybir
    from contextlib import ExitStack

    f32 = mybir.dt.float32
    bf = mybir.dt.bfloat16
    nc = _mk_nc()
    xT = nc.dram_tensor("xT", [IN_C, NPC], bf, kind="ExternalInput").ap()
    w1 = nc.dram_tensor("w1", [IN_C, HID], bf, kind="ExternalInput").ap()
    b1 = nc.dram_tensor("b1", [HID, 1], f32, kind="ExternalInput").ap()
    hT_out = nc.dram_tensor("hT_out", [HID, NPC], bf, kind="ExternalOutput").ap()

    with tile.TileContext(nc) as tc, ExitStack() as ctx:
        wpool = ctx.enter_context(tc.tile_pool(name="w", bufs=1))
        xpool = ctx.enter_context(tc.tile_pool(name="x", bufs=1))
        hpool = ctx.enter_context(tc.tile_pool(name="h", bufs=1))
        psum_pool = ctx.enter_context(tc.tile_pool(name="ps", bufs=2, space="PSUM"))

        w1a = _load_w(nc, wpool, w1[0:128, :], 128, HID, "w1a", bf)
        w1b = _load_w(nc, wpool, w1[128:256, :], 128, HID, "w1b", bf)
        b1t = _load_w(nc, wpool, b1, HID, 1, "b1t")
        xa = xpool.tile([128, NPC], bf, tag="xa")
        xb = xpool.tile([128, NPC], bf, tag="xb")
        nc.sync.dma_start(xa[:], xT[0:128, :])
        nc.sync.dma_start(xb[:], xT[128:256, :])

        hT = hpool.tile([HID, NPC], bf)
        NCH = 500
        Relu = mybir.ActivationFunctionType.Relu
        for j0 in range(0, NPC, NCH):
            w = min(NCH, NPC - j0)
            ps = psum_pool.tile([HID, NCH], f32, tag="p1")
            nc.tensor.matmul(out=ps[:, :w], lhsT=w1a[:], rhs=xa[:, j0 : j0 + w], start=True, stop=False)
            nc.tensor.matmul(out=ps[:, :w], lhsT=w1b[:], rhs=xb[:, j0 : j0 + w], start=False, stop=True)
            nc.scalar.activation(hT[:, j0 : j0 + w], ps[:, :w], Relu, bias=b1t[:])
            nc.gpsimd.dma_start(hT_out[:, j0 : j0 + w], hT[:, j0 : j0 + w])
    _split_multi_waits(nc)
    return nc


# ---------------------------------------------------------------- launch B (layer 1)
def _build_launch_B(tt, chunks):
    import concourse.tile as tile
    from concourse import mybir
    from contextlib import ExitStack

    f32 = mybir.dt.float32
    bf = mybir.dt.bfloat16
    AT = mybir.AluOpType
    Relu = mybir.ActivationFunctionType.Relu
    Ident = mybir.ActivationFunctionType.Identity
    nc = _mk_nc()

    ed_d = nc.dram_tensor("ed", [128, tt, HID], bf, kind="ExternalInput").ap()
    dj8_d = nc.dram_tensor("dj8", [128, tt, 8], bf, kind="ExternalInput").ap()
    hT_d = nc.dram_tensor("hT", [HID, NPC], bf, kind="ExternalInput").ap()
    dinv_nm_d = nc.dram_tensor("dinv_nm", [128, NSB], f32, kind="ExternalInput").ap()
    s_row_d = nc.dram_tensor("s_row", [1, NPC], bf, kind="ExternalInput").ap()
    bv0r_d = nc.dram_tensor("bv0r", [1, HID], bf, kind="ExternalInput").ap()
    wv0_d = nc.dram_tensor("wv0", [HID, HID], bf, kind="ExternalInput").ap()
    wk2_d = nc.dram_tensor("wk2", [HID, HID], bf, kind="ExternalInput").ap()
    wv2_d = nc.dram_tensor("wv2", [HID, HID], bf, kind="ExternalInput").ap()
    wq2_d = nc.dram_tensor("wq2", [HID, HID], bf, kind="ExternalInput").ap()
    bq2_d = nc.dram_tensor("bq2", [HID, 1], f32, kind="ExternalInput").ap()
    o1T_d = nc.dram_tensor("o1T", [HID, NPC], bf, kind="ExternalOutput").ap()
    kd_d = nc.dram_tensor("kd", [HID, NPC], bf, kind="ExternalOutput").ap()
    dv_d = nc.dram_tensor("dv", [HID, NPC], bf, kind="ExternalOutput").ap()
    v1s_d = nc.dram_tensor("v1s", [NPC, HID], bf, kind="ExternalOutput").ap()
    q2T_d = nc.dram_tensor("q2T", [HID, NPC], f32, kind="ExternalOutput").ap()

    with tile.TileContext(nc) as tc, ExitStack() as ctx:
        ident_f, ident_b = _consts(nc, tc, ctx)
        ed_pool = ctx.enter_context(tc.tile_pool(name="ed", bufs=3))
        pref = []
        for (b, t0, gw, first, last) in chunks[:1]:
            edp = ed_pool.tile([128, MAXG, HID], bf, tag="ed")
            nc.gpsimd.dma_start(edp[:, :gw], ed_d[:, t0 : t0 + gw, :])
            pref.append(edp)
        meta_pool = ctx.enter_context(tc.tile_pool(name="meta", bufs=1))
        dj8_t = meta_pool.tile([128, tt, 8], bf, tag="dj8")
        nc.sync.dma_start(dj8_t[:], dj8_d[:])
        wpool = ctx.enter_context(tc.tile_pool(name="w", bufs=1))
        hpool = ctx.enter_context(tc.tile_pool(name="h", bufs=1))
        tr_pool = ctx.enter_context(tc.tile_pool(name="tr", bufs=2))
        st_pool = ctx.enter_context(tc.tile_pool(name="st", bufs=2))
        tab_pool = ctx.enter_context(tc.tile_pool(name="tab", bufs=2))
        psum_seg = ctx.enter_context(tc.tile_pool(name="pseg", bufs=2, space="PSUM"))
        psum_t = ctx.enter_context(tc.tile_pool(name="pt", bufs=1, space="PSUM"))
        psum_m = ctx.enter_context(tc.tile_pool(name="pm", bufs=2, space="PSUM"))

        wv0t = _load_w(nc, wpool, wv0_d, HID, HID, "wv0t", bf)
        wk2t = _load_w(nc, wpool, wk2_d, HID, HID, "wk2t", bf)
        wv2t = _load_w(nc, wpool, wv2_d, HID, HID, "wv2t", bf)
        wq2t = _load_w(nc, wpool, wq2_d, HID, HID, "wq2t", bf)
        bq2t = _load_w(nc, wpool, bq2_d, HID, 1, "bq2t")
        dinv_nm = _load_w(nc, wpool, dinv_nm_d, 128, NSB, "dinv")
        s_row = _load_w(nc, wpool, s_row_d, 1, NPC, "srow", bf)
        bv0r = _load_w(nc, wpool, bv0r_d, 1, HID, "bv0r", bf)
        hT = hpool.tile([HID, NPC], bf, tag="hT")
        nc.sync.dma_start(hT[:], hT_d[:])
        o1T = hpool.tile([HID, NPC], bf, tag="o1T")

        # h-side layer-2 tables (independent of o1; overlaps edge ramp)
        khT = hpool.tile([HID, NPC], bf, tag="khT")
        _proj_sbuf(nc, wk2t, hT, khT, psum_m)
        vhT = hpool.tile([HID, NPC], bf, tag="vhT")
        _proj_sbuf(nc, wv2t, hT, vhT, psum_m)

        def group_tables(b):
            # tables for sbs [b, b+3] in one 512-col batch
            j0 = b * SBT
            wg = min(4 * SBT, NPC - j0)
            o1g = o1T[:, j0 : j0 + wg]
            nc.sync.dma_start(o1T_d[:, j0 : j0 + wg], o1g)
            ko1 = psum_m.tile([HID, 512], f32, tag="mt")
            nc.tensor.matmul(out=ko1[:, :wg], lhsT=wk2t[:], rhs=o1g, start=True, stop=True)
            kdc = tab_pool.tile([HID, 512], bf, tag="kdc")
            nc.vector.tensor_tensor(out=kdc[:, :wg], in0=khT[:, j0 : j0 + wg], in1=ko1[:, :wg], op=AT.subtract)
            nc.sync.dma_start(kd_d[:, j0 : j0 + wg], kdc[:, :wg])
            vo1 = psum_m.tile([HID, 512], f32, tag="mt")
            nc.tensor.matmul(out=vo1[:, :wg], lhsT=wv2t[:], rhs=o1g, start=True, stop=True)
            dvc = tab_pool.tile([HID, 512], bf, tag="dvc")
            nc.vector.tensor_tensor(out=dvc[:, :wg], in0=vhT[:, j0 : j0 + wg], in1=vo1[:, :wg], op=AT.subtract)
            nc.sync.dma_start(dv_d[:, j0 : j0 + wg], dvc[:, :wg])
            # v1s rows: dinv_j-scaled transposes of v1 = wv2.T o1
            v1g = tab_pool.tile([HID, 512], bf, tag="v1g")
            nc.scalar.copy(v1g[:, :wg], vo1[:, :wg])
            for k in range((wg + SBT - 1) // SBT):
                wk_ = min(SBT, wg - k * SBT)
                pv = psum_t.tile([128, HID], bf, tag="pv")
                nc.tensor.transpose(
                    out=pv[:wk_], in_=v1g[:, k * SBT : k * SBT + wk_],
                    identity=ident_b[:HID, :HID],
                )
                v1s = tab_pool.tile([128, HID], bf, tag="v1s")
                nc.scalar.activation(
                    v1s[:wk_], pv[:wk_], Ident, scale=dinv_nm[:wk_, b + k : b + k + 1]
                )
                nc.sync.dma_start(v1s_d[j0 + k * SBT : j0 + k * SBT + wk_, :], v1s[:wk_])
            # q2 columns
            q2 = psum_m.tile([HID, 512], f32, tag="mt")
            nc.tensor.matmul(out=q2[:, :wg], lhsT=wq2t[:], rhs=o1g, start=True, stop=True)
            q2s = tab_pool.tile([HID, 512], f32, tag="q2s")
            nc.scalar.activation(q2s[:, :wg], q2[:, :wg], Ident, bias=bq2t[:])
            nc.sync.dma_start(q2T_d[:, j0 : j0 + wg], q2s[:, :wg])

        def out_cb(b, psT):
            j0, w = _sb_w(b)
            st = st_pool.tile([128, HID], bf, tag="st")
            nc.scalar.activation(st[:w], psT[:w], Ident, scale=dinv_nm[:w, b : b + 1])
            pt = psum_t.tile([HID, 128], bf, tag="pt")
            nc.tensor.transpose(out=pt[:, :w], in_=st[:w], identity=ident_b[:w, :w])
            stT = st_pool.tile([HID, 128], bf, tag="stT")
            nc.scalar.copy(stT[:, :w], pt[:, :w])
            ps3 = psum_t.tile([HID, 128], f32, tag="ps3")
            nc.tensor.matmul(out=ps3[:, :w], lhsT=wv0t[:], rhs=stT[:, :w], start=True, stop=False)
            nc.tensor.matmul(
                out=ps3[:, :w], lhsT=bv0r[:], rhs=s_row[:, j0 : j0 + w],
                start=False, stop=True,
            )
            o1c = o1T[:, j0 : j0 + w]
            nc.scalar.activation(o1c, ps3[:, :w], Relu)
            if b % 4 == 0:
                group_tables(b)

        state = {"ps": None, "started": False, "pend_cb": None}

        def emit_mm(rhs, stop=False):
            nc.tensor.matmul(
                out=state["ps"][:], lhsT=ident_b[:], rhs=rhs,
                start=(not state["started"]), stop=stop,
            )
            state["started"] = True

        def process(b, t0, gw, first, last, ed_t, hs):
            if first:
                ps_new = psum_seg.tile([128, HID], f32, tag="ps")
                state["ps"] = ps_new
                state["started"] = False
            # tree-reduce the chunk's tiles; odd leftovers go straight to PE
            cur = hs
            width = gw
            level = 0
            mms = []
            while width > 1:
                if width % 2 == 1:
                    mms.append(cur[:, width - 1])
                    width -= 1
                half = width // 2
                nxt = tr_pool.tile([128, MAXG // 2, HID], bf, tag=f"tr{level}")
                nc.vector.tensor_tensor(
                    out=nxt[:, :half], in0=cur[:, 0:half], in1=cur[:, half:width],
                    op=AT.add,
                )
                cur = nxt
                width = half
                level += 1
            mms.append(cur[:, 0])
            for i, rhs in enumerate(mms):
                emit_mm(rhs, stop=(last and i == len(mms) - 1))
            if last:
                if state.get("pend_cb") is not None:
                    out_cb(*state["pend_cb"])
                state["pend_cb"] = (b, state["ps"])

        pend = None
        for ci, (b, t0, gw, first, last) in enumerate(chunks):
            if ci < 1:
                ed_t = pref[ci]
            else:
                ed_t = ed_pool.tile([128, MAXG, HID], bf, tag="ed")
                nc.gpsimd.dma_start(ed_t[:, :gw], ed_d[:, t0 : t0 + gw, :])
            hs = ed_pool.tile([128, MAXG, HID], bf, tag="hs")
            nc.vector.tensor_tensor(
                out=hs[:, :gw].rearrange("p c (d h) -> p c d h", d=8),
                in0=ed_t[:, :gw].rearrange("p c (d h) -> p c d h", d=8),
                in1=dj8_t[:, t0 : t0 + gw, None, :].to_broadcast([128, gw, 8, 8]),
                op=AT.mult,
            )
            if pend is not None:
                process(*pend)
            pend = (b, t0, gw, first, last, ed_t, hs)
        process(*pend)
        if state.get("pend_cb") is not None:
            out_cb(*state["pend_cb"])
    _split_multi_waits(nc)
    return nc


# ---------------------------------------------------------------- launch C (layer 2)
def _build_launch_C(tt, tps):
    import concourse.tile as tile
    from concourse import mybir
    from contextlib import ExitStack

    f32 = mybir.dt.float32
    bf = mybir.dt.bfloat16
    AT = mybir.AluOpType
    Relu = mybir.ActivationFunctionType.Relu
    Ident = mybir.ActivationFunctionType.Identity
    Sig = mybir.ActivationFunctionType.Sigmoid
    nc = _mk_nc()
    roww = 3 * HID  # 192: [kd | dv | v1s]
    CH = 64
    plan = _plan_super(tps, CH)

    ed_d = nc.dram_tensor("ed", [128, tt, roww], bf, kind="ExternalInput").ap()
    dj8_d = nc.dram_tensor("dj8", [128, tt, 8], bf, kind="ExternalInput").ap()
    qT_d = nc.dram_tensor("qT", [HID, NPC], f32, kind="ExternalInput").ap()
    hT_d = nc.dram_tensor("hT", [HID, NPC], bf, kind="ExternalInput").ap()
    o1T_d = nc.dram_tensor("o1T", [HID, NPC], bf, kind="ExternalInput").ap()
    dinv_nm_d = nc.dram_tensor("dinv_nm", [128, NSB], f32, kind="ExternalInput").ap()
    sd_row_d = nc.dram_tensor("sd_row", [1, NPC], bf, kind="ExternalInput").ap()
    bv2r_d = nc.dram_tensor("bv2r", [1, HID], bf, kind="ExternalInput").ap()
    wk3_d = nc.dram_tensor("wk3", [HID, HID], bf, kind="ExternalInput").ap()
    wv3_d = nc.dram_tensor("wv3", [HID, HID], bf, kind="ExternalInput").ap()
    wq3_d = nc.dram_tensor("wq3", [HID, HID], bf, kind="ExternalInput").ap()
    bq3_d = nc.dram_tensor("bq3", [HID, 1], f32, kind="ExternalInput").ap()
    kd0_d = nc.dram_tensor("kd0", [HID, NPC], bf, kind="ExternalOutput").ap()
    kd1_d = nc.dram_tensor("kd1", [HID, NPC], bf, kind="ExternalOutput").ap()
    vd0_d = nc.dram_tensor("vd0", [HID, NPC], bf, kind="ExternalOutput").ap()
    vd1_d = nc.dram_tensor("vd1", [HID, NPC], bf, kind="ExternalOutput").ap()
    v2s_d = nc.dram_tensor("v2s", [NPC, HID], bf, kind="ExternalOutput").ap()
    q3T_d = nc.dram_tensor("q3T", [HID, NPC], f32, kind="ExternalOutput").ap()

    with tile.TileContext(nc) as tc, ExitStack() as ctx:
        ident_f, ident_b = _consts(nc, tc, ctx)
        ed_pool = ctx.enter_context(tc.tile_pool(name="ed", bufs=4))
        pref = []
        for (t0, gw, segs) in plan[:1]:
            edp = ed_pool.tile([128, CH, roww], bf, tag="ed")
            nc.gpsimd.dma_start(edp[:, :gw], ed_d[:, t0 : t0 + gw, :])
            pref.append(edp)
        meta_pool = ctx.enter_context(tc.tile_pool(name="meta", bufs=1))
        dj8_t = meta_pool.tile([128, tt, 8], bf, tag="dj8")
        nc.sync.dma_start(dj8_t[:], dj8_d[:])
        qrows = _qrows_from_cols(nc, tc, ctx, qT_d, ident_f)
        wpool = ctx.enter_context(tc.tile_pool(name="w", bufs=1))
        hpool = ctx.enter_context(tc.tile_pool(name="h", bufs=1))
        wk3t = _load_w(nc, wpool, wk3_d, HID, HID, "wk3t", bf)
        wv3t = _load_w(nc, wpool, wv3_d, HID, HID, "wv3t", bf)
        wq3t = _load_w(nc, wpool, wq3_d, HID, HID, "wq3t", bf)
        bq3t = _load_w(nc, wpool, bq3_d, HID, 1, "bq3t")
        dinv_nm = _load_w(nc, wpool, dinv_nm_d, 128, NSB, "dinv")
        sd_row = _load_w(nc, wpool, sd_row_d, 1, NPC, "sdrow", bf)
        bv2r = _load_w(nc, wpool, bv2r_d, 1, HID, "bv2r", bf)
        hT = hpool.tile([HID, NPC], bf, tag="hT")
        nc.sync.dma_start(hT[:], hT_d[:])
        o1T = hpool.tile([HID, NPC], bf, tag="o1T")
        nc.sync.dma_start(o1T[:], o1T_d[:])
        o2T = hpool.tile([HID, NPC], bf, tag="o2T")
        dk_pool = ctx.enter_context(tc.tile_pool(name="dk", bufs=1))
        sc_pool = ctx.enter_context(tc.tile_pool(name="sc", bufs=2))
        msg_pool = ctx.enter_context(tc.tile_pool(name="msg", bufs=2))
        st_pool = ctx.enter_context(tc.tile_pool(name="st", bufs=2))
        tab_pool = ctx.enter_context(tc.tile_pool(name="tab", bufs=1))
        psum_seg = ctx.enter_context(tc.tile_pool(name="pseg", bufs=2, space="PSUM"))
        psum_t = ctx.enter_context(tc.tile_pool(name="pt", bufs=1, space="PSUM"))
        psum_m = ctx.enter_context(tc.tile_pool(name="pm", bufs=2, space="PSUM"))

        def stageA(ed_t, segs, t0, gw):
            # z = q . kd  (d-major tree), a0 = sigmoid(z)
            dk = dk_pool.tile([128, CH, HID], bf, tag="dk")
            for (b, off, w, fi, la) in segs:
                nc.vector.tensor_tensor(
                    out=dk[:, off : off + w], in0=ed_t[:, off : off + w, 0:HID],
                    in1=qrows[:, b : b + 1, :].to_broadcast([128, w, HID]),
                    op=AT.mult,
                )
            dk5 = dk[:, :gw].rearrange("p c (d h) -> p c d h", d=8)
            r4 = sc_pool.tile([128, CH, 4, 8], bf, tag="r4")
            nc.vector.tensor_tensor(
                out=r4[:, :gw], in0=dk5[:, :, 0:4], in1=dk5[:, :, 4:8], op=AT.add
            )
            r2 = sc_pool.tile([128, CH, 2, 8], bf, tag="r2")
            nc.vector.tensor_tensor(
                out=r2[:, :gw], in0=r4[:, :gw, 0:2], in1=r4[:, :gw, 2:4], op=AT.add
            )
            z = sc_pool.tile([128, CH, 8], bf, tag="z")
            nc.vector.tensor_tensor(
                out=z[:, :gw], in0=r2[:, :gw, 0], in1=r2[:, :gw, 1], op=AT.add
            )
            a0 = sc_pool.tile([128, CH, 8], bf, tag="a0")
            nc.scalar.activation(a0[:, :gw], z[:, :gw], Sig)
            return (ed_t, a0, t0, gw)

        def stageB(aobj):
            (ed_t, a0, t0, gw) = aobj
            a0s = sc_pool.tile([128, CH, 8], bf, tag="a0s")
            nc.vector.tensor_tensor(
                out=a0s[:, :gw], in0=a0[:, :gw], in1=dj8_t[:, t0 : t0 + gw],
                op=AT.mult,
            )
            dv = ed_t[:, :gw, HID : 2 * HID].rearrange("p c (d h) -> p c d h", d=8)
            mdv = msg_pool.tile([128, CH, 8, 8], bf, tag="mdv")
            nc.vector.tensor_tensor(
                out=mdv[:, :gw], in0=dv,
                in1=a0s[:, :gw, None, :].to_broadcast([128, gw, 8, 8]),
                op=AT.mult,
            )
            # msum = a0*dinvj*dv + v1s  (v1s already dinv_j-scaled)
            msum = msg_pool.tile([128, CH, HID], bf, tag="msum")
            nc.vector.tensor_tensor(
                out=msum[:, :gw].rearrange("p c (d h) -> p c d h", d=8),
                in0=mdv[:, :gw],
                in1=ed_t[:, :gw, 2 * HID : 3 * HID].rearrange(
                    "p c (d h) -> p c d h", d=8
                ),
                op=AT.add,
            )
            return (msum, gw)

        def group_tables(b):
            # tables for sbs [b, b+3] in one 512-col batch; biases cancel in
            # the differences so project the diffs directly
            j0 = b * SBT
            wg = min(4 * SBT, NPC - j0)
            o2g = o2T[:, j0 : j0 + wg]
            hmo2 = tab_pool.tile([HID, 512], bf, tag="hmo2")
            nc.vector.tensor_tensor(out=hmo2[:, :wg], in0=hT[:, j0 : j0 + wg], in1=o2g, op=AT.subtract)
            o1mo2 = tab_pool.tile([HID, 512], bf, tag="o1mo2")
            nc.vector.tensor_tensor(out=o1mo2[:, :wg], in0=o1T[:, j0 : j0 + wg], in1=o2g, op=AT.subtract)
            kp0 = psum_m.tile([HID, 512], f32, tag="mt")
            nc.tensor.matmul(out=kp0[:, :wg], lhsT=wk3t[:], rhs=hmo2[:, :wg], start=True, stop=True)
            kd0c = tab_pool.tile([HID, 512], bf, tag="kd0c")
            nc.scalar.activation(kd0c[:, :wg], kp0[:, :wg], Ident)
            nc.sync.dma_start(kd0_d[:, j0 : j0 + wg], kd0c[:, :wg])
            kp1 = psum_m.tile([HID, 512], f32, tag="mt")
            nc.tensor.matmul(out=kp1[:, :wg], lhsT=wk3t[:], rhs=o1mo2[:, :wg], start=True, stop=True)
            kd1c = tab_pool.tile([HID, 512], bf, tag="kd1c")
            nc.scalar.activation(kd1c[:, :wg], kp1[:, :wg], Ident)
            nc.sync.dma_start(kd1_d[:, j0 : j0 + wg], kd1c[:, :wg])
            vp0 = psum_m.tile([HID, 512], f32, tag="mt")
            nc.tensor.matmul(out=vp0[:, :wg], lhsT=wv3t[:], rhs=hmo2[:, :wg], start=True, stop=True)
            vd0c = tab_pool.tile([HID, 512], bf, tag="vd0c")
            nc.scalar.activation(vd0c[:, :wg], vp0[:, :wg], Ident)
            nc.sync.dma_start(vd0_d[:, j0 : j0 + wg], vd0c[:, :wg])
            vp1 = psum_m.tile([HID, 512], f32, tag="mt")
            nc.tensor.matmul(out=vp1[:, :wg], lhsT=wv3t[:], rhs=o1mo2[:, :wg], start=True, stop=True)
            vd1c = tab_pool.tile([HID, 512], bf, tag="vd1c")
            nc.scalar.activation(vd1c[:, :wg], vp1[:, :wg], Ident)
            nc.sync.dma_start(vd1_d[:, j0 : j0 + wg], vd1c[:, :wg])
            vo2 = psum_m.tile([HID, 512], f32, tag="mt")
            nc.tensor.matmul(out=vo2[:, :wg], lhsT=wv3t[:], rhs=o2g, start=True, stop=True)
            # v2s rows: dinv_j-scaled transposes of v2 = wv3.T o2
            v2g = tab_pool.tile([HID, 512], bf, tag="v2g")
            nc.scalar.copy(v2g[:, :wg], vo2[:, :wg])
            for k in range((wg + SBT - 1) // SBT):
                wk_ = min(SBT, wg - k * SBT)
                pv = psum_t.tile([128, HID], bf, tag="pv")
                nc.tensor.transpose(
                    out=pv[:wk_], in_=v2g[:, k * SBT : k * SBT + wk_],
                    identity=ident_b[:HID, :HID],
                )
                v2s = tab_pool.tile([128, HID], bf, tag="v2s")
                nc.scalar.activation(
                    v2s[:wk_], pv[:wk_], Ident, scale=dinv_nm[:wk_, b + k : b + k + 1]
                )
                nc.sync.dma_start(v2s_d[j0 + k * SBT : j0 + k * SBT + wk_, :], v2s[:wk_])
            # q3 columns
            q3 = psum_m.tile([HID, 512], f32, tag="mt")
            nc.tensor.matmul(out=q3[:, :wg], lhsT=wq3t[:], rhs=o2g, start=True, stop=True)
            q3s = tab_pool.tile([HID, 512], f32, tag="q3s")
            nc.scalar.activation(q3s[:, :wg], q3[:, :wg], Ident, bias=bq3t[:])
            nc.sync.dma_start(q3T_d[:, j0 : j0 + wg], q3s[:, :wg])

        def out_cb(b, psT):
            j0, w = _sb_w(b)
            st = st_pool.tile([128, HID], bf, tag="st")
            nc.scalar.activation(st[:w], psT[:w], Relu, scale=dinv_nm[:w, b : b + 1])
            pt = psum_t.tile([HID, 128], bf, tag="pt")
            nc.tensor.transpose(out=pt[:, :w], in_=st[:w], identity=ident_b[:w, :w])
            o2c = o2T[:, j0 : j0 + w]
            nc.scalar.copy(o2c, pt[:, :w])
            if b % 4 == 0:
                group_tables(b)

        state = {"ps": None, "pend_cb": None}

        def finish(segs, bobj):
            (msum, gw) = bobj
            for (b, off, w, fi, la) in segs:
                if fi:
                    ps_new = psum_seg.tile([128, HID], f32, tag="ps")
                    state["ps"] = ps_new
                ps = state["ps"]
                for i in range(w):
                    nc.tensor.matmul(
                        out=ps[:], lhsT=ident_b[:], rhs=msum[:, off + i],
                        start=(fi and i == 0), stop=False,
                    )
                if la:
                    j0, w2 = _sb_w(b)
                    nc.tensor.matmul(
                        out=ps[:w2], lhsT=sd_row[:, j0 : j0 + w2], rhs=bv2r[:],
                        start=False, stop=True,
                    )
                    if state["pend_cb"] is not None:
                        out_cb(*state["pend_cb"])
                    state["pend_cb"] = (b, ps)

        pendA = None
        for ci, (t0, gw, segs) in enumerate(plan):
            if ci < 1:
                ed_t = pref[ci]
            else:
                ed_t = ed_pool.tile([128, CH, roww], bf, tag="ed")
                nc.gpsimd.dma_start(ed_t[:, :gw], ed_d[:, t0 : t0 + gw, :])
            aobj = stageA(ed_t, segs, t0, gw)
            if pendA is not None:
                (psegs, paobj) = pendA
                finish(psegs, stageB(paobj))
            pendA = (segs, aobj)
        (psegs, paobj) = pendA
        finish(psegs, stageB(paobj))
        if state["pend_cb"] is not None:
            out_cb(*state["pend_cb"])
    _split_multi_waits(nc)
    return nc


# ---------------------------------------------------------------- launch D (layer 3 + head)
def _build_launch_D(tt, tps):
    import concourse.tile as tile
    from concourse import mybir
    from contextlib import ExitStack

    f32 = mybir.dt.float32
    bf = mybir.dt.bfloat16
    AT = mybir.AluOpType
    Exp = mybir.ActivationFunctionType.Exp
    Ln = mybir.ActivationFunctionType.Ln
    nc = _mk_nc()
    roww = 5 * HID  # 320: [kd0 | kd1 | vd0 | vd1 | v2s]
    CH = 40
    plan = _plan_super(tps, CH)

    ed_d = nc.dram_tensor("ed", [128, tt, roww], bf, kind="ExternalInput").ap()
    dj8_d = nc.dram_tensor("dj8", [128, tt, 8], bf, kind="ExternalInput").ap()
    qT_d = nc.dram_tensor("qT", [HID, NPC], f32, kind="ExternalInput").ap()
    dinv_nm_d = nc.dram_tensor("dinv_nm", [128, NSB], f32, kind="ExternalInput").ap()
    sd_row_d = nc.dram_tensor("sd_row", [1, NPC], bf, kind="ExternalInput").ap()
    bv3r_d = nc.dram_tensor("bv3r", [1, HID], bf, kind="ExternalInput").ap()
    w2_d = nc.dram_tensor("w2", [HID, OUT_C], bf, kind="ExternalInput").ap()
    b2bc_d = nc.dram_tensor("b2bc", [128, OUT_C], f32, kind="ExternalInput").ap()
    y_d = nc.dram_tensor("y", [NPC, OUT_C], f32, kind="ExternalOutput").ap()

    with tile.TileContext(nc) as tc, ExitStack() as ctx:
        ident_f, ident_b = _consts(nc, tc, ctx)
        ed_pool = ctx.enter_context(tc.tile_pool(name="ed", bufs=3))
        pref = []
        for (t0, gw, segs) in plan[:1]:
            edp = ed_pool.tile([128, CH, roww], bf, tag="ed")
            nc.gpsimd.dma_start(edp[:, :gw], ed_d[:, t0 : t0 + gw, :])
            pref.append(edp)
        meta_pool = ctx.enter_context(tc.tile_pool(name="meta", bufs=1))
        dj8_t = meta_pool.tile([128, tt, 8], bf, tag="dj8")
        nc.sync.dma_start(dj8_t[:], dj8_d[:])
        qrows = _qrows_from_cols(nc, tc, ctx, qT_d, ident_f)
        wpool = ctx.enter_context(tc.tile_pool(name="w", bufs=1))
        w2t = _load_w(nc, wpool, w2_d, HID, OUT_C, "w2t", bf)
        b2t = _load_w(nc, wpool, b2bc_d, 128, OUT_C, "b2t")
        dinv_nm = _load_w(nc, wpool, dinv_nm_d, 128, NSB, "dinv")
        sd_row = _load_w(nc, wpool, sd_row_d, 1, NPC, "sdrow", bf)
        bv3r = _load_w(nc, wpool, bv3r_d, 1, HID, "bv3r", bf)
        dk_pool = ctx.enter_context(tc.tile_pool(name="dk", bufs=2))
        sc_pool = ctx.enter_context(tc.tile_pool(name="sc", bufs=2))
        msg_pool = ctx.enter_context(tc.tile_pool(name="msg", bufs=2))
        st_pool = ctx.enter_context(tc.tile_pool(name="st", bufs=2))
        sm_pool = ctx.enter_context(tc.tile_pool(name="sm", bufs=2))
        psum_seg = ctx.enter_context(tc.tile_pool(name="pseg", bufs=2, space="PSUM"))
        psum_t = ctx.enter_context(tc.tile_pool(name="pt", bufs=2, space="PSUM"))
        psum_lg = ctx.enter_context(tc.tile_pool(name="plg", bufs=2, space="PSUM"))

        def stageA(ed_t, segs, t0, gw):
            # z_t = q . kd_t (d-major tree), ee = exp(z),
            # rz = 1/(1+e0+e1) = exp(-ln(1 + e0 + e1))  (Exp/Ln only)
            dk = dk_pool.tile([128, CH, 2, HID], bf, tag="dk")
            for (b, off, w, fi, la) in segs:
                nc.vector.tensor_tensor(
                    out=dk[:, off : off + w],
                    in0=ed_t[:, off : off + w, 0 : 2 * HID].rearrange(
                        "p c (t d) -> p c t d", t=2
                    ),
                    in1=qrows[:, b : b + 1, None, :].to_broadcast([128, w, 2, HID]),
                    op=AT.mult,
                )
            dk5 = dk[:, :gw].rearrange("p c t (d h) -> p c t d h", d=8)
            r4 = sc_pool.tile([128, CH, 2, 4, 8], bf, tag="r4")
            nc.vector.tensor_tensor(
                out=r4[:, :gw], in0=dk5[:, :, :, 0:4], in1=dk5[:, :, :, 4:8], op=AT.add
            )
            r2 = sc_pool.tile([128, CH, 2, 2, 8], bf, tag="r2")
            nc.vector.tensor_tensor(
                out=r2[:, :gw], in0=r4[:, :gw, :, 0:2], in1=r4[:, :gw, :, 2:4], op=AT.add
            )
            sc = sc_pool.tile([128, CH, 2, 8], bf, tag="sc")
            nc.vector.tensor_tensor(
                out=sc[:, :gw], in0=r2[:, :gw, :, 0], in1=r2[:, :gw, :, 1], op=AT.add
            )
            ee = sc_pool.tile([128, CH, 2, 8], bf, tag="ee")
            nc.scalar.activation(ee[:, :gw], sc[:, :gw], Exp)
            dd = sc_pool.tile([128, CH, 8], bf, tag="dd")
            nc.vector.tensor_tensor(
                out=dd[:, :gw], in0=ee[:, :gw, 0], in1=ee[:, :gw, 1], op=AT.add
            )
            lse = sc_pool.tile([128, CH, 8], f32, tag="lse")
            nc.scalar.activation(lse[:, :gw], dd[:, :gw], Ln, bias=1.0)
            rz = sc_pool.tile([128, CH, 8], bf, tag="rz")
            nc.scalar.activation(rz[:, :gw], lse[:, :gw], Exp, scale=-1.0)
            return (ed_t, ee, rz, t0, gw)

        def stageB(mobj):
            (ed_t, ee, rz, t0, gw) = mobj
            rzs = sc_pool.tile([128, CH, 8], bf, tag="rzs")
            nc.vector.tensor_tensor(
                out=rzs[:, :gw], in0=rz[:, :gw], in1=dj8_t[:, t0 : t0 + gw],
                op=AT.mult,
            )
            aa = sc_pool.tile([128, CH, 2, 8], bf, tag="aa")
            nc.vector.tensor_tensor(
                out=aa[:, :gw], in0=ee[:, :gw],
                in1=rzs[:, :gw, None, :].to_broadcast([128, gw, 2, 8]),
                op=AT.mult,
            )
            ve = ed_t[:, :gw, 2 * HID : 4 * HID].rearrange(
                "p c (t d h) -> p c t d h", t=2, d=8
            )
            mm = msg_pool.tile([128, CH, 2, 8, 8], bf, tag="mm")
            nc.vector.tensor_tensor(
                out=mm[:, :gw], in0=ve,
                in1=aa[:, :gw, :, None, :].to_broadcast([128, gw, 2, 8, 8]),
                op=AT.mult,
            )
            m01 = msg_pool.tile([128, CH, 8, 8], bf, tag="m01")
            nc.vector.tensor_tensor(
                out=m01[:, :gw], in0=mm[:, :gw, 0], in1=mm[:, :gw, 1], op=AT.add
            )
            # msum = m01 + v2s  (v2s carries dinv_j; its attn weight sums to 1)
            msum = msg_pool.tile([128, CH, HID], bf, tag="msum")
            nc.vector.tensor_tensor(
                out=msum[:, :gw].rearrange("p c (d h) -> p c d h", d=8),
                in0=m01[:, :gw],
                in1=ed_t[:, :gw, 4 * HID : 5 * HID].rearrange(
                    "p c (d h) -> p c d h", d=8
                ),
                op=AT.add,
            )
            return (msum, gw)

        def out_cb(b, psT):
            j0, w = _sb_w(b)
            # relu + dinv_i scale on the DVE (keeps ScalarE's ACT table small)
            st = st_pool.tile([128, HID], bf, tag="st")
            nc.vector.tensor_scalar(
                out=st[:w], in0=psT[:w], scalar1=dinv_nm[:w, b : b + 1],
                scalar2=0.0, op0=AT.mult, op1=AT.max,
            )
            pt = psum_t.tile([HID, 128], bf, tag="pt")
            nc.tensor.transpose(out=pt[:, :w], in_=st[:w], identity=ident_b[:w, :w])
            o3T = st_pool.tile([HID, 128], bf, tag="o3T")
            nc.scalar.copy(o3T[:, :w], pt[:, :w])
            lg = psum_lg.tile([128, OUT_C], f32, tag="lg")
            nc.tensor.matmul(out=lg[:w], lhsT=o3T[:, :w], rhs=w2t[:], start=True, stop=True)
            logits = sm_pool.tile([128, OUT_C], f32, tag="logits")
            nc.vector.tensor_tensor(out=logits[:w], in0=lg[:w], in1=b2t[:w], op=AT.add)
            nlmax = sm_pool.tile([128, 1], f32, tag="nlmax")
            nc.vector.tensor_reduce(
                out=nlmax[:w], in_=logits[:w], axis=mybir.AxisListType.X,
                op=AT.max, negate=True,
            )
            eb = sm_pool.tile([128, OUT_C], f32, tag="eb")
            esum = sm_pool.tile([128, 1], f32, tag="esum")
            nc.scalar.activation(
                eb[:w], logits[:w], Exp, bias=nlmax[:w], accum_out=esum[:w]
            )
            lse2 = sm_pool.tile([128, 1], f32, tag="lse2")
            nc.scalar.activation(lse2[:w], esum[:w], Ln)
            off = sm_pool.tile([128, 1], f32, tag="off")
            nc.vector.tensor_tensor(out=off[:w], in0=lse2[:w], in1=nlmax[:w], op=AT.subtract)
            yy = sm_pool.tile([128, OUT_C], f32, tag="yy")
            nc.vector.tensor_tensor(
                out=yy[:w], in0=logits[:w],
                in1=off[:w].to_broadcast([w, OUT_C]), op=AT.subtract,
            )
            nc.sync.dma_start(y_d[j0 : j0 + w, :], yy[:w])

        state = {"ps": None, "pend_cb": None}

        def finish(segs, bobj):
            (msum, gw) = bobj
            for (b, off, w, fi, la) in segs:
                if fi:
                    ps_new = psum_seg.tile([128, HID], f32, tag="ps")
                    state["ps"] = ps_new
                ps = state["ps"]
                for i in range(w):
                    nc.tensor.matmul(
                        out=ps[:], lhsT=ident_b[:], rhs=msum[:, off + i],
                        start=(fi and i == 0), stop=False,
                    )
                if la:
                    j0, w2 = _sb_w(b)
                    nc.tensor.matmul(
                        out=ps[:w2], lhsT=sd_row[:, j0 : j0 + w2], rhs=bv3r[:],
                        start=False, stop=True,
                    )
                    if state["pend_cb"] is not None:
                        out_cb(*state["pend_cb"])
                    state["pend_cb"] = (b, ps)

        pendA = None
        for ci, (t0, gw, segs) in enumerate(plan):
            if ci < 1:
                ed_t = pref[ci]
            else:
                ed_t = ed_pool.tile([128, CH, roww], bf, tag="ed")
                nc.gpsimd.dma_start(ed_t[:, :gw], ed_d[:, t0 : t0 + gw, :])
            aobj = stageA(ed_t, segs, t0, gw)
            if pendA is not None:
                (psegs, paobj) = pendA
                finish(psegs, stageB(paobj))
            pendA = (segs, aobj)
        (psegs, paobj) = pendA
        finish(psegs, stageB(paobj))
        if state["pend_cb"] is not None:
            out_cb(*state["pend_cb"])
    _split_multi_waits(nc)
    return nc


# ---------------------------------------------------------------- host gather
def _u16(a):
    return a.view(np.uint16)


def _gather_cat(tabs, eidx):
    """[128, TT, sum(w)] bf16: rows gathered by global src id from each
    table [N+1, w] (row N is the zero sentinel), concatenated."""
    tt = eidx.shape[1]
    ws = [t.shape[1] for t in tabs]
    out = np.empty((128, tt, sum(ws)), dtype=np.uint16)
    o = 0
    for t, w in zip(tabs, ws):
        out[:, :, o : o + w] = _u16(t)[eidx]
        o += w
    return out.view(BF16)


def _scatter_cols(cols_list, ids):
    """tab[global_id] = cols.T for each core; sentinel zero row."""
    w = cols_list[0].shape[0]
    tab = np.zeros((N + 1, w), dtype=BF16)
    for c in range(NCORES):
        tab[ids[c]] = cols_list[c].T
    return tab


def _scatter_rows(rows_list, ids):
    w = rows_list[0].shape[1]
    tab = np.zeros((N + 1, w), dtype=BF16)
    for c in range(NCORES):
        tab[ids[c]] = rows_list[c]
    return tab


# ---------------------------------------------------------------- driver
def kernel(x, edge_index, lin1_w, lin1_b, wq, bq, wk, bk, wv, bv, lin2_w, lin2_b):
    _install_fixups()
    from concourse.bass_utils import run_bass_kernel_spmd

    x = np.asarray(x, dtype=np.float32)
    lin1_w = np.asarray(lin1_w, np.float32)
    lin1_b = np.asarray(lin1_b, np.float32)
    wq = np.asarray(wq, np.float32)
    bq = np.asarray(bq, np.float32)
    wk = np.asarray(wk, np.float32)
    wv = np.asarray(wv, np.float32)
    bv = np.asarray(bv, np.float32)
    lin2_w = np.asarray(lin2_w, np.float32)
    lin2_b = np.asarray(lin2_b, np.float32)
    isd = np.float32(1.0 / np.sqrt(DH))

    metas, tps, tt, chunks, s_all, dinv, ids = _preprocess(np.asarray(edge_index))

    key = ("progs", tps, tt)
    if key not in _CACHE:
        _CACHE[key] = (
            _build_launch_A(),
            _build_launch_B(tt, chunks),
            _build_launch_C(tt, tps),
            _build_launch_D(tt, tps),
        )
    ncA, ncB, ncC, ncD = _CACHE[key]
    cores = list(range(NCORES))

    def conj(W):  # d-major conjugation
        return W[PRM][:, PRM]

    # per-core metadata columns
    dinv_nm = []
    s_rows = []
    sd_rows = []
    for c in cores:
        dv = dinv[ids[c]]
        dm = np.ones((128, NSB), np.float32)
        for b in range(NSB):
            j0, w = b * SBT, min(SBT, NPC - b * SBT)
            dm[:w, b] = dv[j0 : j0 + w]
        dinv_nm.append(dm)
        s_rows.append(s_all[ids[c]][None, :].astype(BF16))
        sd_rows.append((s_all[ids[c]] / dv)[None, :].astype(BF16))

    # ---- launch A: h = relu(x @ W1 + b1) (PRM basis), columnar bf16
    xT = np.ascontiguousarray(x.T).astype(BF16)
    w1_bf = lin1_w[:, PRM].astype(BF16)
    b1_prm = lin1_b[PRM][:, None]
    in_maps = [
        dict(
            xT=np.ascontiguousarray(xT[:, ids[c]]),
            w1=w1_bf,
            b1=b1_prm,
        )
        for c in cores
    ]
    resA = run_bass_kernel_spmd(ncA, in_maps, cores)
    hT = [np.asarray(resA.results[c]["hT_out"]) for c in cores]
    h_tab = _scatter_cols(hT, ids)

    # ---- launch B: layer 1 (attn == identity) + kd/dv/v1s/q2 tables
    in_maps = [
        dict(
            ed=_gather_cat([h_tab], metas[c]["eidx"]),
            dj8=metas[c]["dj8"],
            hT=hT[c],
            dinv_nm=dinv_nm[c],
            s_row=s_rows[c],
            bv0r=bv[0][PRM][None, :].astype(BF16),
            wv0=conj(wv[0]).astype(BF16),
            wk2=conj(wk[1]).astype(BF16),
            wv2=conj(wv[1]).astype(BF16),
            wq2=(conj(wq[1]) * isd).astype(BF16),
            bq2=(bq[1][PRM] * isd)[:, None],
        )
        for c in cores
    ]
    resB = run_bass_kernel_spmd(ncB, in_maps, cores)
    o1T = [np.asarray(resB.results[c]["o1T"]) for c in cores]
    q2T = [np.asarray(resB.results[c]["q2T"]) for c in cores]
    kd_tab = _scatter_cols([np.asarray(resB.results[c]["kd"]) for c in cores], ids)
    dv_tab = _scatter_cols([np.asarray(resB.results[c]["dv"]) for c in cores], ids)
    v1s_tab = _scatter_rows([np.asarray(resB.results[c]["v1s"]) for c in cores], ids)

    # ---- launch C: layer 2 + kd0/kd1/vd0/vd1/v2s/q3 tables
    in_maps = [
        dict(
            ed=_gather_cat([kd_tab, dv_tab, v1s_tab], metas[c]["eidx"]),
            dj8=metas[c]["dj8"],
            qT=q2T[c],
            hT=hT[c],
            o1T=o1T[c],
            dinv_nm=dinv_nm[c],
            sd_row=sd_rows[c],
            bv2r=bv[1][PRM][None, :].astype(BF16),
            wk3=conj(wk[2]).astype(BF16),
            wv3=conj(wv[2]).astype(BF16),
            wq3=(conj(wq[2]) * isd).astype(BF16),
            bq3=(bq[2][PRM] * isd)[:, None],
        )
        for c in cores
    ]
    resC = run_bass_kernel_spmd(ncC, in_maps, cores)
    q3T = [np.asarray(resC.results[c]["q3T"]) for c in cores]
    kd0_tab = _scatter_cols([np.asarray(resC.results[c]["kd0"]) for c in cores], ids)
    kd1_tab = _scatter_cols([np.asarray(resC.results[c]["kd1"]) for c in cores], ids)
    vd0_tab = _scatter_cols([np.asarray(resC.results[c]["vd0"]) for c in cores], ids)
    vd1_tab = _scatter_cols([np.asarray(resC.results[c]["vd1"]) for c in cores], ids)
    v2s_tab = _scatter_rows([np.asarray(resC.results[c]["v2s"]) for c in cores], ids)

    # ---- launch D: layer 3 + classifier head + log_softmax
    b2bc = np.ascontiguousarray(np.broadcast_to(lin2_b[None, :], (128, OUT_C)))
    in_maps = [
        dict(
            ed=_gather_cat(
                [kd0_tab, kd1_tab, vd0_tab, vd1_tab, v2s_tab], metas[c]["eidx"]
            ),
            dj8=metas[c]["dj8"],
            qT=q3T[c],
            dinv_nm=dinv_nm[c],
            sd_row=sd_rows[c],
            bv3r=bv[2][PRM][None, :].astype(BF16),
            w2=lin2_w[PRM, :].astype(BF16),
            b2bc=b2bc,
        )
        for c in cores
    ]
    resD = run_bass_kernel_spmd(ncD, in_maps, cores)
    y = np.empty((N, OUT_C), dtype=np.float32)
    for c in cores:
        y[ids[c]] = np.asarray(resD.results[c]["y"], dtype=np.float32)
    return y
